# revision 1
# baseline (speedup 1.0000x reference)
"""Two-layer dropout-masked LSTM (B=512, T=256, I=64, H0=128, H1=32) on 8 trn2 cores.

Data-parallel over batch: 64 rows/core. State kept transposed [feature, batch].
Per-step PSUM bank layout (512 f32 cols): [i0|f0|o0|g0'|i1|f1|o1|g1'] where the
L1 block holds the *previous* step's layer-1 gates (L1 runs one iteration behind
L0 so sigmoids cover whole banks). tanh(g) = 2*sigmoid(2g) - 1 with the 2x
pre-scaled into the g-gate weights, so ACT only runs Sigmoid + one Tanh(c) per
step. Fused scalar_tensor_tensor keeps DVE at ~7 ops/step.

The toolchain's walrus build supports at most ONE semaphore wait per
instruction, so the program is structured to never need two: all static data
(weights/bias/x/masks) arrives via five upfront DMAs, masks live in SBUF as
uint8 {0,1} with the dropout scale folded into fused scalar_tensor_tensor ops
(no per-step DMAs at all), state inits run on DVE, an ACT preamble absorbs the
const-DMA tick + loads the sigmoid/tanh table set, and a tiny per-group PE
dummy matmul absorbs the PSUM-slot WAR tick.
"""

import numpy as np

B, T, I, H0, H1 = 512, 256, 64, 128, 32
NCORES = 8
BS = B // NCORES  # 64
# col-block order within a PSUM step-bank: i, f, o, g (pytorch rows are i,f,g,o)
GATE_ORDER = [0, 1, 3, 2]
G_GATE = 2  # pytorch block index of the tanh gate, pre-scaled by 2

# packed-constant tensor column layout
C_WIH0 = 0        # rows 0:65,  cols 0:512
C_WHH0 = 512      # rows 0:128, cols 512:1024
C_WIH1 = 1024     # rows 0:128, cols 1024:1152
C_WHH1 = 1152     # rows 0:33,  cols 1152:1280
C_WLIN = 1280     # rows 0:33,  col 1280
C_BIAS = 1281     # zeros col
C_COLS = 1282
MASK_SCALE = float(np.float32(1.0) / np.float32(1.0 - 0.4))

_CACHED = {}


def _build_program(debug_steps=(), n_steps=T):
    import os
    import concourse.bass as bass
    import concourse.tile as tile
    from concourse import mybir
    from contextlib import ExitStack

    ABL = set(os.environ.get("K_ABLATE", "").split(","))

    f32 = mybir.dt.float32
    u8 = mybir.dt.uint8
    AF = mybir.ActivationFunctionType
    ALU = mybir.AluOpType

    nc = bass.Bass()

    MCOLS = (T + 1) * 128
    cst_d = nc.declare_dram_parameter("cst", [128, C_COLS], f32, isOutput=False)
    xt_d = nc.declare_dram_parameter("xt", [I + 1, T * BS], f32, isOutput=False)
    mph_d = nc.declare_dram_parameter("mph", [128, MCOLS], u8, isOutput=False)
    mpc_d = nc.declare_dram_parameter("mpc", [128, MCOLS], u8, isOutput=False)
    y_d = nc.declare_dram_parameter("y", [BS, 1], f32, isOutput=True)
    dbg_d = {}
    for dt_ in debug_steps:
        dbg_d[dt_] = {
            name: nc.declare_dram_parameter(f"dbg_{name}_{dt_}", shape, f32, isOutput=True)
            for name, shape in (
                ("S", [128, 512]), ("vc", [128, 128]), ("c", [128, 128]),
                ("T", [128, 128]), ("o2", [128, 128]), ("h0", [H0, BS]),
                ("h1", [H1 + 1, BS]), ("bank", [128, 512]),
            )
        }

    GRP = 4  # timesteps per PSUM tile (4 banks); bufs=2 -> all 8 banks

    with ExitStack() as ctx:
        tc = ctx.enter_context(tile.TileContext(nc))
        const = ctx.enter_context(tc.tile_pool(name="const", bufs=1))
        xpool = ctx.enter_context(tc.tile_pool(name="xtp", bufs=1))
        psum = ctx.enter_context(
            tc.tile_pool(name="gates", bufs=2, space=bass.MemorySpace.PSUM)
        )
        spool = ctx.enter_context(tc.tile_pool(name="sig", bufs=2))
        mpool = ctx.enter_context(tc.tile_pool(name="masks", bufs=1))
        wpool = ctx.enter_context(tc.tile_pool(name="work", bufs=2))
        hpool = ctx.enter_context(tc.tile_pool(name="state", bufs=2))

        cst = const.tile([128, C_COLS], f32)
        nc.sync.dma_start(cst[:], cst_d[:])
        xt = xpool.tile([I + 1, T * BS], f32)
        nc.sync.dma_start(xt[:], xt_d[:])
        mph = mpool.tile([128, MCOLS], u8, tag="mph")
        nc.sync.dma_start(mph[:], mph_d[:])
        mpc = mpool.tile([128, MCOLS], u8, tag="mpc")
        nc.sync.dma_start(mpc[:], mpc_d[:])

        bias0 = cst[:, C_BIAS : C_BIAS + 1]

        # ACT preamble: absorb the cst DMA tick on ACT and preload the
        # sigmoid/tanh table set before the timestep loop.
        scratch = const.tile([128, 1], f32)
        nc.scalar.activation(scratch[:], cst[:, 0:1], AF.Copy)
        nc.scalar.activation(scratch[:], cst[:, 0:1], AF.Sigmoid, bias=bias0)
        nc.scalar.activation(scratch[:], cst[:, 0:1], AF.Tanh, bias=bias0)
        # DVE preamble: absorb the two mask DMA ticks so in-loop mask reads
        # never add a second wait on top of same-engine pipeline waits.
        scr8 = const.tile([1, 2], u8)
        nc.vector.tensor_copy(scr8[0:1, 0:1], mph[0:1, 0:1])
        nc.vector.tensor_copy(scr8[0:1, 1:2], mpc[0:1, 0:1])

        # ---- initial state (DVE so consumers' waits stay single-source) ----
        h0_prev = hpool.tile([H0, BS], f32, tag="h0")
        nc.vector.memset(h0_prev[:], 0.0)
        h1_slot0 = hpool.tile([H1 + 1, BS], f32, tag="h1")
        nc.vector.memset(h1_slot0[H1 : H1 + 1, :], 1.0)
        h1_prev = hpool.tile([H1 + 1, BS], f32, tag="h1")
        nc.vector.memset(h1_prev[0:H1, :], 0.0)
        nc.vector.memset(h1_prev[H1 : H1 + 1, :], 1.0)
        c_prev = wpool.tile([128, 128], f32, tag="c")
        nc.vector.memset(c_prev[:], 0.0)

        pt = None
        ptv = None
        for t in range(n_steps + 1):
            k, s = divmod(t, GRP)
            if s == 0:
                pt = psum.tile([128, GRP * 512], f32, tag="gates")
                ptv = pt[:].rearrange("p (s c) -> p s c", s=GRP)
                if "dummy" not in ABL:
                    # tiny dummy matmul: absorbs the PSUM-slot WAR (ACT sigma
                    # readers from group k-2) onto PE before any real writer.
                    nc.tensor.matmul(
                        ptv[0:1, GRP - 1, 256:257],
                        cst[0:1, 0:1],
                        cst[0:1, 0:1],
                        start=True,
                        stop=True,
                    )

            if t < n_steps and "rec" not in ABL:
                # L0 gates for step t: x-part (bias via ones row) + recurrent,
                # as immediately-paired accumulation groups (interleaving
                # start=True groups with deferred start=False continuations
                # corrupts PSUM on this toolchain).
                for j in range(4):
                    nc.tensor.matmul(
                        ptv[:, s, j * 64 : (j + 1) * 64],
                        cst[0 : I + 1, C_WIH0 + j * 128 : C_WIH0 + (j + 1) * 128],
                        xt[:, t * BS : (t + 1) * BS],
                        start=True,
                        stop=False,
                    )
                    nc.tensor.matmul(
                        ptv[:, s, j * 64 : (j + 1) * 64],
                        cst[0:H0, C_WHH0 + j * 128 : C_WHH0 + (j + 1) * 128],
                        h0_prev[:],
                        start=False,
                        stop=True,
                    )
            if t >= 1 and "l1" not in ABL:
                # L1 gates for step t-1 (uses h0_{t-1}, h1_{t-2}; bias via ones row)
                for j in range(4):
                    nc.tensor.matmul(
                        ptv[0:H1, s, 256 + j * 64 : 256 + (j + 1) * 64],
                        cst[0:H0, C_WIH1 + j * H1 : C_WIH1 + (j + 1) * H1],
                        h0_prev[:],
                        start=True,
                        stop=False,
                    )
                    nc.tensor.matmul(
                        ptv[0:H1, s, 256 + j * 64 : 256 + (j + 1) * 64],
                        cst[0 : H1 + 1, C_WHH1 + j * H1 : C_WHH1 + (j + 1) * H1],
                        h1_prev[:],
                        start=False,
                        stop=True,
                    )

            # ---- sigmoids over the whole bank ----
            S = spool.tile([128, 512], f32, tag="S")
            if t < n_steps:
                nc.scalar.activation(
                    S[:, 0:256], ptv[:, s, 0:256], AF.Sigmoid, bias=bias0
                )
            if t >= 1:
                nc.scalar.activation(
                    S[0:H1, 256:512],
                    ptv[0:H1, s, 256:512],
                    AF.Sigmoid,
                    bias=bias0[0:H1, :],
                )

            # 3D views: [128, 2 blocks, 64]; block 0 = L0 step t, block 1 = L1 step t-1
            Sv = S[:].rearrange("p (a c) -> p a c", a=2)
            si = Sv[:, :, 0:64]
            sf = Sv[:, :, 64:128]
            so = Sv[:, :, 128:192]
            sg = Sv[:, :, 192:256]

            Pp = wpool.tile([128, 128], f32, tag="Pp")
            Ppv = Pp[:].rearrange("p (a c) -> p a c", a=2)
            v = wpool.tile([128, 128], f32, tag="v")
            vv = v[:].rearrange("p (a c) -> p a c", a=2)
            vc = wpool.tile([128, 128], f32, tag="vc")
            vcv = vc[:].rearrange("p (a c) -> p a c", a=2)
            c_new = wpool.tile([128, 128], f32, tag="c")
            cnv = c_new[:].rearrange("p (a c) -> p a c", a=2)
            cpv = c_prev[:].rearrange("p (a c) -> p a c", a=2)
            Mcv = mpc[:, t * 128 : (t + 1) * 128].rearrange("p (a c) -> p a c", a=2)
            Mhv = mph[:, t * 128 : (t + 1) * 128].rearrange("p (a c) -> p a c", a=2)

            # P' = (sig(g') - 0.5) * sig(i)   [= tanh(g)*sig(i)/2]
            nc.vector.scalar_tensor_tensor(Ppv, sg, 0.5, si, ALU.subtract, ALU.mult)
            # v = sig(f) * c_prev
            nc.vector.tensor_tensor(vv, sf, cpv, ALU.mult)
            # vc = 2*P' + v
            nc.vector.scalar_tensor_tensor(vcv, Ppv, 2.0, vv, ALU.mult, ALU.add)
            # c = (vc * dropout_scale) * mask_c_u8
            nc.vector.scalar_tensor_tensor(cnv, vcv, MASK_SCALE, Mcv, ALU.mult, ALU.mult)
            if t == 0:
                # layer-1 half of the c state must start at zero (kills psum junk)
                nc.vector.memset(c_new[:, 64:128], 0.0)

            Tt = wpool.tile([128, 128], f32, tag="T")
            # h uses tanh of the UNMASKED cell state (mask only hits the carry)
            nc.scalar.activation(Tt[:], vc[:], AF.Tanh, bias=bias0)

            o2 = wpool.tile([128, 128], f32, tag="o2")
            o2v = o2[:].rearrange("p (a c) -> p a c", a=2)
            # o'' = (sig(o) * dropout_scale) * mask_h_u8
            nc.vector.scalar_tensor_tensor(o2v, so, MASK_SCALE, Mhv, ALU.mult, ALU.mult)

            if t < n_steps:
                h0_new = hpool.tile([H0, BS], f32, tag="h0")
                nc.vector.tensor_tensor(h0_new[:], o2[:, 0:64], Tt[:, 0:64], ALU.mult)
                h0_prev = h0_new
            if t >= 1:
                h1_new = hpool.tile([H1 + 1, BS], f32, tag="h1")
                nc.vector.tensor_tensor(
                    h1_new[0:H1, :], o2[0:H1, 64:128], Tt[0:H1, 64:128], ALU.mult
                )
                h1_prev = h1_new
            c_prev = c_new
            if t in dbg_d:
                dd = dbg_d[t]
                bank_sb = spool.tile([128, 512], f32, tag="bankdbg")
                nc.scalar.copy(bank_sb[:], ptv[:, s, :])
                nc.sync.dma_start(dd["bank"][:], bank_sb[:])
                nc.sync.dma_start(dd["S"][:], S[:])
                nc.sync.dma_start(dd["vc"][:], vc[:])
                nc.sync.dma_start(dd["c"][:], c_new[:])
                nc.sync.dma_start(dd["T"][:], Tt[:])
                nc.sync.dma_start(dd["o2"][:], o2[:])
                nc.sync.dma_start(dd["h0"][:], h0_prev[:])
                nc.sync.dma_start(dd["h1"][:], h1_prev[:])

        # ---- final projection: y = h1_255 @ W_lin.T + b_lin  -> [64, 1] ----
        yp = ptv[0:BS, 1, 0:1]
        nc.tensor.matmul(
            yp, h1_prev[:], cst[0 : H1 + 1, C_WLIN : C_WLIN + 1], start=True, stop=True
        )
        y_sb = const.tile([BS, 1], f32)
        nc.scalar.copy(y_sb[:], yp)
        nc.sync.dma_start(y_d[:], y_sb[:])

    _split_multiwaits(nc)
    return nc


def _split_multiwaits(nc):
    """This toolchain's walrus accepts at most one semaphore wait per
    instruction. Split any extra waits onto standalone EventSemaphore
    instructions inserted just before the offending instruction on the same
    engine queue (in-order execution preserves semantics exactly)."""
    from concourse import mybir

    n = 0
    for fn in nc.m.functions:
        for bb in fn.blocks:
            out = []
            for inst in bb.instructions:
                si = inst.sync_info
                if si is not None and si.on_wait and len(si.on_wait) > 1:
                    waits = list(si.on_wait)
                    for w in waits[:-1]:
                        n += 1
                        out.append(
                            mybir.InstEventSemaphore(
                                name=f"I-wsplit-{n}",
                                engine=inst.engine,
                                ins=[],
                                outs=[],
                                sync_info=mybir.SyncInfo(on_wait=[w], on_update=[]),
                            )
                        )
                    inst.sync_info = mybir.SyncInfo(
                        on_wait=[waits[-1]], on_update=list(si.on_update)
                    )
                out.append(inst)
            bb.instructions = out


def _prep_consts(W_ih0, W_hh0, b_ih0, b_hh0, W_ih1, W_hh1, b_ih1, b_hh1, W_lin, b_lin):
    f = np.float32
    b0 = (b_ih0 + b_hh0).astype(f)
    b1 = (b_ih1 + b_hh1).astype(f)
    cst = np.zeros((128, C_COLS), f)
    for j, g in enumerate(GATE_ORDER):
        m = 2.0 if g == G_GATE else 1.0
        cst[0:I, C_WIH0 + j * H0 : C_WIH0 + (j + 1) * H0] = (
            m * W_ih0[g * H0 : (g + 1) * H0].T
        )
        cst[I, C_WIH0 + j * H0 : C_WIH0 + (j + 1) * H0] = m * b0[g * H0 : (g + 1) * H0]
        cst[0:H0, C_WHH0 + j * H0 : C_WHH0 + (j + 1) * H0] = (
            m * W_hh0[g * H0 : (g + 1) * H0].T
        )
        cst[0:H0, C_WIH1 + j * H1 : C_WIH1 + (j + 1) * H1] = (
            m * W_ih1[g * H1 : (g + 1) * H1].T
        )
        cst[0:H1, C_WHH1 + j * H1 : C_WHH1 + (j + 1) * H1] = (
            m * W_hh1[g * H1 : (g + 1) * H1].T
        )
        cst[H1, C_WHH1 + j * H1 : C_WHH1 + (j + 1) * H1] = m * b1[g * H1 : (g + 1) * H1]
    cst[0:H1, C_WLIN] = W_lin[0]
    cst[H1, C_WLIN] = b_lin[0]
    return cst


def kernel(
    input_seq,
    mask_h0,
    mask_c0,
    mask_h1,
    mask_c1,
    W_ih0,
    W_hh0,
    b_ih0,
    b_hh0,
    W_ih1,
    W_hh1,
    b_ih1,
    b_hh1,
    W_lin,
    b_lin,
):
    import sys

    for p in ("/opt/trn_rl_repo",):
        if p not in sys.path:
            sys.path.insert(0, p)
    from concourse.bass_utils import run_bass_kernel_spmd

    f = np.float32
    input_seq = np.asarray(input_seq, f)
    mask_h0 = np.asarray(mask_h0, f)
    mask_c0 = np.asarray(mask_c0, f)
    mask_h1 = np.asarray(mask_h1, f)
    mask_c1 = np.asarray(mask_c1, f)
    args = [np.asarray(a, f) for a in (W_ih0, W_hh0, b_ih0, b_hh0,
                                       W_ih1, W_hh1, b_ih1, b_hh1, W_lin, b_lin)]
    cst = _prep_consts(*args)

    in_maps = []
    for c in range(NCORES):
        lo, hi = c * BS, (c + 1) * BS
        xs = input_seq[lo:hi]  # [BS, T, I]
        xt = np.empty((I + 1, T * BS), f)
        xt[0:I] = xs.transpose(2, 1, 0).reshape(I, T * BS)
        xt[I] = 1.0
        u8 = np.uint8
        mph3 = np.zeros((T + 1, 128, 128), u8)
        mpc3 = np.zeros((T + 1, 128, 128), u8)
        mph3[0:T, :, 0:64] = (mask_h0[:, lo:hi, :] != 0).transpose(0, 2, 1)
        mph3[1 : T + 1, 0:H1, 64:128] = (mask_h1[:, lo:hi, :] != 0).transpose(0, 2, 1)
        mpc3[0:T, :, 0:64] = (mask_c0[:, lo:hi, :] != 0).transpose(0, 2, 1)
        mpc3[1 : T + 1, 0:H1, 64:128] = (mask_c1[:, lo:hi, :] != 0).transpose(0, 2, 1)
        mph = np.ascontiguousarray(mph3.transpose(1, 0, 2).reshape(128, -1))
        mpc = np.ascontiguousarray(mpc3.transpose(1, 0, 2).reshape(128, -1))
        in_maps.append({"cst": cst, "xt": xt, "mph": mph, "mpc": mpc})

    if "nc" not in _CACHED:
        _CACHED["nc"] = _build_program()
    _CACHED["in_maps"] = in_maps
    res = run_bass_kernel_spmd(_CACHED["nc"], in_maps, list(range(NCORES)))
    out = np.concatenate([res.results[c]["y"] for c in range(NCORES)], axis=0)
    return out.astype(f)



# revision 7
# speedup vs baseline: 1.8820x; 1.8820x over previous
"""Two-layer dropout-masked LSTM (B=512, T=256, I=64, H0=128, H1=32) on 8 trn2 cores.

Data-parallel over batch: 64 rows/core. State kept transposed [feature, batch].
Per-step PSUM bank layout (512 f32 cols): [i0|f0|o0|g0'|i1|f1|o1|g1'] where the
L1 block holds the *previous* step's layer-1 gates (L1 runs one iteration behind
L0 so sigmoids cover whole banks). tanh(g) = 2*sigmoid(2g) - 1 with the 2x
pre-scaled into the g-gate weights, so ACT only runs Sigmoid + one Tanh(c) per
step. Fused scalar_tensor_tensor keeps DVE at ~7 ops/step.

The toolchain's walrus build supports at most ONE semaphore wait per
instruction, so the program is structured to never need two: all static data
(weights/bias/x/masks) arrives via five upfront DMAs, masks live in SBUF as
uint8 {0,1} with the dropout scale folded into fused scalar_tensor_tensor ops
(no per-step DMAs at all), state inits run on DVE, an ACT preamble absorbs the
const-DMA tick + loads the sigmoid/tanh table set, and a tiny per-group PE
dummy matmul absorbs the PSUM-slot WAR tick.
"""

import numpy as np

B, T, I, H0, H1 = 512, 256, 64, 128, 32
NCORES = 8
BS = B // NCORES  # 64
# col-block order within a PSUM step-bank: i, f, o, g (pytorch rows are i,f,g,o)
GATE_ORDER = [0, 1, 3, 2]
G_GATE = 2  # pytorch block index of the tanh gate, pre-scaled by 2

# packed-constant tensor column layout
C_WIH0 = 0        # rows 0:65,  cols 0:512
C_WHH0 = 512      # rows 0:128, cols 512:1024
C_WIH1 = 1024     # rows 0:128, cols 1024:1152
C_WHH1 = 1152     # rows 0:33,  cols 1152:1280
C_WLIN = 1280     # rows 0:33,  col 1280
C_BIAS = 1281     # zeros col
C_COLS = 1282
MASK_SCALE = float(np.float32(1.0) / np.float32(1.0 - 0.4))

_CACHED = {}


def _build_program(debug_steps=(), n_steps=T):
    import os
    import concourse.bass as bass
    import concourse.tile as tile
    from concourse import mybir
    from contextlib import ExitStack

    ABL = set(os.environ.get("K_ABLATE", "").split(","))

    f32 = mybir.dt.float32
    bf16 = mybir.dt.bfloat16
    u8 = mybir.dt.uint8
    AF = mybir.ActivationFunctionType
    ALU = mybir.AluOpType

    nc = bass.Bass()

    MCOLS = (T + 1) * 128
    cst_d = nc.declare_dram_parameter("cst", [128, C_COLS], bf16, isOutput=False)
    xt_d = nc.declare_dram_parameter("xt", [I + 1, T * BS], bf16, isOutput=False)
    mph_d = nc.declare_dram_parameter("mph", [128, MCOLS], u8, isOutput=False)
    mpc_d = nc.declare_dram_parameter("mpc", [128, MCOLS], u8, isOutput=False)
    y_d = nc.declare_dram_parameter("y", [BS, 1], f32, isOutput=True)
    dbg_d = {}
    for dt_ in debug_steps:
        dbg_d[dt_] = {
            name: nc.declare_dram_parameter(f"dbg_{name}_{dt_}", shape, f32, isOutput=True)
            for name, shape in (
                ("S", [128, 512]), ("vc", [128, 128]), ("c", [128, 128]),
                ("T", [128, 128]), ("o2", [128, 128]), ("h0", [H0, BS]),
                ("h1", [H1 + 1, BS]), ("bank", [128, 512]),
            )
        }

    GRP = 4  # timesteps per PSUM tile (4 banks); bufs=2 -> all 8 banks

    with ExitStack() as ctx:
        tc = ctx.enter_context(tile.TileContext(nc))
        const = ctx.enter_context(tc.tile_pool(name="const", bufs=1))
        xpool = ctx.enter_context(tc.tile_pool(name="xtp", bufs=1))
        psum = ctx.enter_context(
            tc.tile_pool(name="gates", bufs=2, space=bass.MemorySpace.PSUM)
        )
        spool = ctx.enter_context(tc.tile_pool(name="sig", bufs=2))
        mpool = ctx.enter_context(tc.tile_pool(name="masks", bufs=1))
        wpool = ctx.enter_context(tc.tile_pool(name="work", bufs=2))
        hpool = ctx.enter_context(tc.tile_pool(name="state", bufs=2))

        cst = const.tile([128, C_COLS], bf16)
        nc.sync.dma_start(cst[:], cst_d[:])
        xt = xpool.tile([I + 1, T * BS], bf16)
        nc.sync.dma_start(xt[:], xt_d[:])
        mph = mpool.tile([128, MCOLS], u8, tag="mph")
        nc.sync.dma_start(mph[:], mph_d[:])
        mpc = mpool.tile([128, MCOLS], u8, tag="mpc")
        nc.sync.dma_start(mpc[:], mpc_d[:])

        bias0 = cst[:, C_BIAS : C_BIAS + 1]

        # ACT preamble: absorb the cst DMA tick on ACT and preload the
        # sigmoid/tanh table set before the timestep loop.
        scratch = const.tile([128, 1], f32)
        nc.scalar.activation(scratch[:], cst[:, 0:1], AF.Copy)
        nc.scalar.activation(scratch[:], cst[:, 0:1], AF.Sigmoid, bias=bias0)
        nc.scalar.activation(scratch[:], cst[:, 0:1], AF.Tanh, bias=bias0)
        # DVE preamble: absorb the two mask DMA ticks so in-loop mask reads
        # never add a second wait on top of same-engine pipeline waits.
        scr8 = const.tile([1, 2], u8)
        nc.vector.tensor_copy(scr8[0:1, 0:1], mph[0:1, 0:1])
        nc.vector.tensor_copy(scr8[0:1, 1:2], mpc[0:1, 0:1])

        # ---- initial state (DVE so consumers' waits stay single-source) ----
        h0_prev = hpool.tile([H0, BS], bf16, tag="h0")
        nc.vector.memset(h0_prev[:], 0.0)
        h1_slot0 = hpool.tile([H1 + 1, BS], bf16, tag="h1")
        nc.vector.memset(h1_slot0[H1 : H1 + 1, :], 1.0)
        h1_prev = hpool.tile([H1 + 1, BS], bf16, tag="h1")
        nc.vector.memset(h1_prev[0:H1, :], 0.0)
        nc.vector.memset(h1_prev[H1 : H1 + 1, :], 1.0)
        c_prev = wpool.tile([128, 128], f32, tag="c")
        nc.vector.memset(c_prev[:], 0.0)

        pt = None
        ptv = None
        for t in range(n_steps + 1):
            k, s = divmod(t, GRP)
            if s == 0:
                pt = psum.tile([128, GRP * 512], f32, tag="gates")
                ptv = pt[:].rearrange("p (s c) -> p s c", s=GRP)
                if "dummy" not in ABL:
                    # tiny dummy matmul: absorbs the PSUM-slot WAR (ACT sigma
                    # readers from group k-2) onto PE before any real writer.
                    nc.tensor.matmul(
                        ptv[0:1, GRP - 1, 256:257],
                        cst[0:1, 0:1],
                        cst[0:1, 0:1],
                        start=True,
                        stop=True,
                    )

            if t < n_steps and "rec" not in ABL:
                # L0 gates for step t: x-part (bias via ones row) + recurrent,
                # as immediately-paired accumulation groups (interleaving
                # start=True groups with deferred start=False continuations
                # corrupts PSUM on this toolchain).
                for j in range(4):
                    nc.tensor.matmul(
                        ptv[:, s, j * 64 : (j + 1) * 64],
                        cst[0 : I + 1, C_WIH0 + j * 128 : C_WIH0 + (j + 1) * 128],
                        xt[:, t * BS : (t + 1) * BS],
                        start=True,
                        stop=False,
                    )
                    nc.tensor.matmul(
                        ptv[:, s, j * 64 : (j + 1) * 64],
                        cst[0:H0, C_WHH0 + j * 128 : C_WHH0 + (j + 1) * 128],
                        h0_prev[:],
                        start=False,
                        stop=True,
                    )
            if t >= 1 and "l1" not in ABL:
                # L1 gates for step t-1 (uses h0_{t-1}, h1_{t-2}; bias via ones row)
                for j in range(4):
                    nc.tensor.matmul(
                        ptv[0:H1, s, 256 + j * 64 : 256 + (j + 1) * 64],
                        cst[0:H0, C_WIH1 + j * H1 : C_WIH1 + (j + 1) * H1],
                        h0_prev[:],
                        start=True,
                        stop=False,
                    )
                    nc.tensor.matmul(
                        ptv[0:H1, s, 256 + j * 64 : 256 + (j + 1) * 64],
                        cst[0 : H1 + 1, C_WHH1 + j * H1 : C_WHH1 + (j + 1) * H1],
                        h1_prev[:],
                        start=False,
                        stop=True,
                    )

            # ---- sigmoids over the whole bank ----
            S = spool.tile([128, 512], f32, tag="S")
            if t < n_steps:
                nc.scalar.activation(
                    S[:, 0:256], ptv[:, s, 0:256], AF.Sigmoid, bias=bias0
                )
            if t >= 1:
                nc.scalar.activation(
                    S[0:H1, 256:512],
                    ptv[0:H1, s, 256:512],
                    AF.Sigmoid,
                    bias=bias0[0:H1, :],
                )

            # 3D views: [128, 2 blocks, 64]; block 0 = L0 step t, block 1 = L1 step t-1
            Sv = S[:].rearrange("p (a c) -> p a c", a=2)
            si = Sv[:, :, 0:64]
            sf = Sv[:, :, 64:128]
            so = Sv[:, :, 128:192]
            sg = Sv[:, :, 192:256]

            Pp = wpool.tile([128, 128], f32, tag="Pp")
            Ppv = Pp[:].rearrange("p (a c) -> p a c", a=2)
            v = wpool.tile([128, 128], f32, tag="v")
            vv = v[:].rearrange("p (a c) -> p a c", a=2)
            vc = wpool.tile([128, 128], f32, tag="vc")
            vcv = vc[:].rearrange("p (a c) -> p a c", a=2)
            c_new = wpool.tile([128, 128], f32, tag="c")
            cnv = c_new[:].rearrange("p (a c) -> p a c", a=2)
            cpv = c_prev[:].rearrange("p (a c) -> p a c", a=2)
            Mcv = mpc[:, t * 128 : (t + 1) * 128].rearrange("p (a c) -> p a c", a=2)
            Mhv = mph[:, t * 128 : (t + 1) * 128].rearrange("p (a c) -> p a c", a=2)

            # P' = (sig(g') - 0.5) * sig(i)   [= tanh(g)*sig(i)/2]
            nc.vector.scalar_tensor_tensor(Ppv, sg, 0.5, si, ALU.subtract, ALU.mult)
            # v = sig(f) * c_prev
            nc.vector.tensor_tensor(vv, sf, cpv, ALU.mult)
            # vc = 2*P' + v
            nc.vector.scalar_tensor_tensor(vcv, Ppv, 2.0, vv, ALU.mult, ALU.add)
            # c = (vc * dropout_scale) * mask_c_u8
            nc.vector.scalar_tensor_tensor(cnv, vcv, MASK_SCALE, Mcv, ALU.mult, ALU.mult)
            if t == 0:
                # layer-1 half of the c state must start at zero (kills psum junk)
                nc.vector.memset(c_new[:, 64:128], 0.0)

            Tt = wpool.tile([128, 128], f32, tag="T")
            # h uses tanh of the UNMASKED cell state (mask only hits the carry)
            nc.scalar.activation(Tt[:], vc[:], AF.Tanh, bias=bias0)

            o2 = wpool.tile([128, 128], f32, tag="o2")
            o2v = o2[:].rearrange("p (a c) -> p a c", a=2)
            # o'' = (sig(o) * dropout_scale) * mask_h_u8
            nc.vector.scalar_tensor_tensor(o2v, so, MASK_SCALE, Mhv, ALU.mult, ALU.mult)

            if t < n_steps:
                h0_new = hpool.tile([H0, BS], bf16, tag="h0")
                nc.vector.tensor_tensor(h0_new[:], o2[:, 0:64], Tt[:, 0:64], ALU.mult)
                h0_prev = h0_new
            if t >= 1:
                h1_new = hpool.tile([H1 + 1, BS], bf16, tag="h1")
                nc.vector.tensor_tensor(
                    h1_new[0:H1, :], o2[0:H1, 64:128], Tt[0:H1, 64:128], ALU.mult
                )
                h1_prev = h1_new
            c_prev = c_new
            if t in dbg_d:
                dd = dbg_d[t]
                bank_sb = spool.tile([128, 512], f32, tag="bankdbg")
                nc.scalar.copy(bank_sb[:], ptv[:, s, :])
                nc.sync.dma_start(dd["bank"][:], bank_sb[:])
                nc.sync.dma_start(dd["S"][:], S[:])
                nc.sync.dma_start(dd["vc"][:], vc[:])
                nc.sync.dma_start(dd["c"][:], c_new[:])
                nc.sync.dma_start(dd["T"][:], Tt[:])
                nc.sync.dma_start(dd["o2"][:], o2[:])
                nc.sync.dma_start(dd["h0"][:], h0_prev[:])
                nc.sync.dma_start(dd["h1"][:], h1_prev[:])

        # ---- final projection: y = h1_255 @ W_lin.T + b_lin  -> [64, 1] ----
        yp = ptv[0:BS, 1, 0:1]
        nc.tensor.matmul(
            yp, h1_prev[:], cst[0 : H1 + 1, C_WLIN : C_WLIN + 1], start=True, stop=True
        )
        y_sb = const.tile([BS, 1], f32)
        nc.scalar.copy(y_sb[:], yp)
        nc.sync.dma_start(y_d[:], y_sb[:])

    _split_multiwaits(nc)
    return nc


def _split_multiwaits(nc):
    """This toolchain's walrus accepts at most one semaphore wait per
    instruction. Split any extra waits onto standalone EventSemaphore
    instructions inserted just before the offending instruction on the same
    engine queue (in-order execution preserves semantics exactly)."""
    from concourse import mybir

    n = 0
    for fn in nc.m.functions:
        for bb in fn.blocks:
            out = []
            for inst in bb.instructions:
                si = inst.sync_info
                if si is not None and si.on_wait and len(si.on_wait) > 1:
                    waits = list(si.on_wait)
                    for w in waits[:-1]:
                        n += 1
                        out.append(
                            mybir.InstEventSemaphore(
                                name=f"I-wsplit-{n}",
                                engine=inst.engine,
                                ins=[],
                                outs=[],
                                sync_info=mybir.SyncInfo(on_wait=[w], on_update=[]),
                            )
                        )
                    inst.sync_info = mybir.SyncInfo(
                        on_wait=[waits[-1]], on_update=list(si.on_update)
                    )
                out.append(inst)
            bb.instructions = out


def _prep_consts(W_ih0, W_hh0, b_ih0, b_hh0, W_ih1, W_hh1, b_ih1, b_hh1, W_lin, b_lin):
    f = np.float32
    b0 = (b_ih0 + b_hh0).astype(f)
    b1 = (b_ih1 + b_hh1).astype(f)
    cst = np.zeros((128, C_COLS), f)
    for j, g in enumerate(GATE_ORDER):
        m = 2.0 if g == G_GATE else 1.0
        cst[0:I, C_WIH0 + j * H0 : C_WIH0 + (j + 1) * H0] = (
            m * W_ih0[g * H0 : (g + 1) * H0].T
        )
        cst[I, C_WIH0 + j * H0 : C_WIH0 + (j + 1) * H0] = m * b0[g * H0 : (g + 1) * H0]
        cst[0:H0, C_WHH0 + j * H0 : C_WHH0 + (j + 1) * H0] = (
            m * W_hh0[g * H0 : (g + 1) * H0].T
        )
        cst[0:H0, C_WIH1 + j * H1 : C_WIH1 + (j + 1) * H1] = (
            m * W_ih1[g * H1 : (g + 1) * H1].T
        )
        cst[0:H1, C_WHH1 + j * H1 : C_WHH1 + (j + 1) * H1] = (
            m * W_hh1[g * H1 : (g + 1) * H1].T
        )
        cst[H1, C_WHH1 + j * H1 : C_WHH1 + (j + 1) * H1] = m * b1[g * H1 : (g + 1) * H1]
    cst[0:H1, C_WLIN] = W_lin[0]
    cst[H1, C_WLIN] = b_lin[0]
    return cst


def kernel(
    input_seq,
    mask_h0,
    mask_c0,
    mask_h1,
    mask_c1,
    W_ih0,
    W_hh0,
    b_ih0,
    b_hh0,
    W_ih1,
    W_hh1,
    b_ih1,
    b_hh1,
    W_lin,
    b_lin,
):
    import sys

    for p in ("/opt/trn_rl_repo",):
        if p not in sys.path:
            sys.path.insert(0, p)
    from concourse.bass_utils import run_bass_kernel_spmd

    import ml_dtypes

    f = np.float32
    bf = ml_dtypes.bfloat16
    input_seq = np.asarray(input_seq, f)
    mask_h0 = np.asarray(mask_h0, f)
    mask_c0 = np.asarray(mask_c0, f)
    mask_h1 = np.asarray(mask_h1, f)
    mask_c1 = np.asarray(mask_c1, f)
    args = [np.asarray(a, f) for a in (W_ih0, W_hh0, b_ih0, b_hh0,
                                       W_ih1, W_hh1, b_ih1, b_hh1, W_lin, b_lin)]
    cst = _prep_consts(*args).astype(bf)

    in_maps = []
    for c in range(NCORES):
        lo, hi = c * BS, (c + 1) * BS
        xs = input_seq[lo:hi]  # [BS, T, I]
        xt = np.empty((I + 1, T * BS), bf)
        xt[0:I] = xs.transpose(2, 1, 0).reshape(I, T * BS).astype(bf)
        xt[I] = 1.0
        u8 = np.uint8
        mph3 = np.zeros((T + 1, 128, 128), u8)
        mpc3 = np.zeros((T + 1, 128, 128), u8)
        mph3[0:T, :, 0:64] = (mask_h0[:, lo:hi, :] != 0).transpose(0, 2, 1)
        mph3[1 : T + 1, 0:H1, 64:128] = (mask_h1[:, lo:hi, :] != 0).transpose(0, 2, 1)
        mpc3[0:T, :, 0:64] = (mask_c0[:, lo:hi, :] != 0).transpose(0, 2, 1)
        mpc3[1 : T + 1, 0:H1, 64:128] = (mask_c1[:, lo:hi, :] != 0).transpose(0, 2, 1)
        mph = np.ascontiguousarray(mph3.transpose(1, 0, 2).reshape(128, -1))
        mpc = np.ascontiguousarray(mpc3.transpose(1, 0, 2).reshape(128, -1))
        in_maps.append({"cst": cst, "xt": xt, "mph": mph, "mpc": mpc})

    if "nc" not in _CACHED:
        _CACHED["nc"] = _build_program()
    _CACHED["in_maps"] = in_maps
    res = run_bass_kernel_spmd(_CACHED["nc"], in_maps, list(range(NCORES)))
    out = np.concatenate([res.results[c]["y"] for c in range(NCORES)], axis=0)
    return out.astype(f)



# revision 15
# speedup vs baseline: 1.9865x; 1.0555x over previous
"""Two-layer dropout-masked LSTM (B=512, T=256, I=64, H0=128, H1=32) on 8 trn2 cores.

Data-parallel over batch: 64 rows/core. State kept transposed [feature, batch].
Per-step PSUM bank layout (512 f32 cols): [i0|f0|o0|g0'|i1|f1|o1|g1'] where the
L1 block holds the *previous* step's layer-1 gates (L1 runs one iteration behind
L0 so sigmoids cover whole banks). tanh(g) = 2*sigmoid(2g) - 1 with the 2x
pre-scaled into the g-gate weights, so ACT only runs Sigmoid + one Tanh(c) per
step. Fused scalar_tensor_tensor keeps DVE at ~7 ops/step.

The toolchain's walrus build supports at most ONE semaphore wait per
instruction, so the program is structured to never need two: all static data
(weights/bias/x/masks) arrives via five upfront DMAs, masks live in SBUF as
uint8 {0,1} with the dropout scale folded into fused scalar_tensor_tensor ops
(no per-step DMAs at all), state inits run on DVE, an ACT preamble absorbs the
const-DMA tick + loads the sigmoid/tanh table set, and a tiny per-group PE
dummy matmul absorbs the PSUM-slot WAR tick.
"""

import numpy as np

B, T, I, H0, H1 = 512, 256, 64, 128, 32
NCORES = 8
BS = B // NCORES  # 64
# col-block order within a PSUM step-bank: i, f, o, g (pytorch rows are i,f,g,o)
GATE_ORDER = [0, 1, 3, 2]
G_GATE = 2  # pytorch block index of the tanh gate, pre-scaled by 2

# packed-constant tensor column layout
C_WIH0 = 0        # rows 0:65,  cols 0:512
C_WHH0 = 512      # rows 0:128, cols 512:1024
C_WIH1 = 1024     # rows 0:128, cols 1024:1152
C_WHH1 = 1152     # rows 0:33,  cols 1152:1280
C_WLIN = 1280     # rows 0:33,  col 1280
C_BIAS = 1281     # zeros col
C_COLS = 1282
MASK_SCALE = float(np.float32(1.0) / np.float32(1.0 - 0.4))

_CACHED = {}


def _build_program(debug_steps=(), n_steps=T):
    import os
    import concourse.bass as bass
    import concourse.tile as tile
    from concourse import mybir
    from contextlib import ExitStack

    ABL = set(os.environ.get("K_ABLATE", "").split(","))

    f32 = mybir.dt.float32
    bf16 = mybir.dt.bfloat16
    u8 = mybir.dt.uint8
    AF = mybir.ActivationFunctionType
    ALU = mybir.AluOpType

    nc = bass.Bass()

    MCOLS = (T + 2) * 128
    cst_d = nc.declare_dram_parameter("cst", [128, C_COLS], bf16, isOutput=False)
    xt_d = nc.declare_dram_parameter("xt", [I + 1, T * BS], bf16, isOutput=False)
    mph_d = nc.declare_dram_parameter("mph", [128, MCOLS], u8, isOutput=False)
    mpc_d = nc.declare_dram_parameter("mpc", [128, MCOLS], u8, isOutput=False)
    y_d = nc.declare_dram_parameter("y", [BS, 1], f32, isOutput=True)
    dbg_d = {}
    for dt_ in debug_steps:
        dbg_d[dt_] = {
            name: nc.declare_dram_parameter(f"dbg_{name}_{dt_}", shape, f32, isOutput=True)
            for name, shape in (
                ("S", [128, 512]), ("vc", [128, 128]), ("c", [128, 128]),
                ("T", [128, 128]), ("o2", [128, 128]), ("h0", [H0, BS]),
                ("h1", [H1 + 1, BS]), ("bank", [128, 512]),
            )
        }

    LAG = 2  # L1 runs LAG iterations behind L0: its matmuls' inputs
    # (h0_{t-LAG}, h1_{t-LAG-1}) are ready early, so they execute during the
    # previous step's ACT/DVE phase instead of blocking the sigmoid.
    GRP = 4  # timesteps per PSUM tile (4 banks); bufs=2 -> all 8 banks

    with ExitStack() as ctx:
        tc = ctx.enter_context(tile.TileContext(nc))
        const = ctx.enter_context(tc.tile_pool(name="const", bufs=1))
        xpool = ctx.enter_context(tc.tile_pool(name="xtp", bufs=1))
        psum = ctx.enter_context(
            tc.tile_pool(name="gates", bufs=2, space=bass.MemorySpace.PSUM)
        )
        spool = ctx.enter_context(tc.tile_pool(name="sig", bufs=2))
        mpool = ctx.enter_context(tc.tile_pool(name="masks", bufs=1))
        wpool = ctx.enter_context(tc.tile_pool(name="work", bufs=2))
        hpool = ctx.enter_context(tc.tile_pool(name="state", bufs=2))
        h0pool = ctx.enter_context(tc.tile_pool(name="state0", bufs=3))

        cst = const.tile([128, C_COLS], bf16)
        nc.sync.dma_start(cst[:], cst_d[:])
        xt = xpool.tile([I + 1, T * BS], bf16)
        nc.sync.dma_start(xt[:], xt_d[:])
        mph = mpool.tile([128, MCOLS], u8, tag="mph")
        nc.sync.dma_start(mph[:], mph_d[:])
        mpc = mpool.tile([128, MCOLS], u8, tag="mpc")
        nc.sync.dma_start(mpc[:], mpc_d[:])

        bias0 = cst[:, C_BIAS : C_BIAS + 1]

        # ACT preamble: absorb the cst DMA tick on ACT and preload the
        # sigmoid/tanh table set before the timestep loop.
        scratch = const.tile([128, 1], f32)
        nc.scalar.activation(scratch[:], cst[:, 0:1], AF.Copy)
        nc.scalar.activation(scratch[:], cst[:, 0:1], AF.Sigmoid, bias=bias0)
        nc.scalar.activation(scratch[:], cst[:, 0:1], AF.Tanh, bias=bias0)
        # DVE preamble: absorb the two mask DMA ticks so in-loop mask reads
        # never add a second wait on top of same-engine pipeline waits.
        scr8 = const.tile([1, 2], u8)
        nc.vector.tensor_copy(scr8[0:1, 0:1], mph[0:1, 0:1])
        nc.vector.tensor_copy(scr8[0:1, 1:2], mpc[0:1, 0:1])

        # ---- initial state (DVE so consumers' waits stay single-source) ----
        h0_prev = h0pool.tile([H0, BS], bf16, tag="h0")
        nc.vector.memset(h0_prev[:], 0.0)
        h0_prev2 = h0_prev
        h1_slot0 = hpool.tile([H1 + 1, BS], bf16, tag="h1")
        nc.vector.memset(h1_slot0[H1 : H1 + 1, :], 1.0)
        h1_prev = hpool.tile([H1 + 1, BS], bf16, tag="h1")
        nc.vector.memset(h1_prev[0:H1, :], 0.0)
        nc.vector.memset(h1_prev[H1 : H1 + 1, :], 1.0)
        c_prev = wpool.tile([128, 128], f32, tag="c")
        nc.vector.memset(c_prev[:], 0.0)

        pt = None
        ptv = None
        for t in range(n_steps + LAG):
            k, s = divmod(t, GRP)
            if s == 0:
                pt = psum.tile([128, GRP * 512], f32, tag="gates")
                ptv = pt[:].rearrange("p (s c) -> p s c", s=GRP)
                if "dummy" not in ABL:
                    # tiny dummy matmul: absorbs the PSUM-slot WAR (ACT sigma
                    # readers from group k-2) onto PE before any real writer.
                    nc.tensor.matmul(
                        ptv[0:1, GRP - 1, 256:257],
                        cst[0:1, 0:1],
                        cst[0:1, 0:1],
                        start=True,
                        stop=True,
                    )

            if t >= LAG and "l1" not in ABL:
                # L1 gates for step t-LAG (h0_{t-LAG}, h1_{t-LAG-1}; bias via
                # ones row). Emitted BEFORE L0 so they sit at the PE queue
                # head with long-satisfied deps and run during the previous
                # step's ACT/DVE phase, off the critical path.
                for j in range(4):
                    nc.tensor.matmul(
                        ptv[0:H1, s, 256 + j * 64 : 256 + (j + 1) * 64],
                        cst[0:H0, C_WIH1 + j * H1 : C_WIH1 + (j + 1) * H1],
                        h0_prev2[:],
                        start=True,
                        stop=False,
                    )
                    nc.tensor.matmul(
                        ptv[0:H1, s, 256 + j * 64 : 256 + (j + 1) * 64],
                        cst[0 : H1 + 1, C_WHH1 + j * H1 : C_WHH1 + (j + 1) * H1],
                        h1_prev[:],
                        start=False,
                        stop=True,
                    )
            if t < n_steps and "rec" not in ABL:
                # L0 gates for step t: x-part (bias via ones row) + recurrent,
                # as immediately-paired accumulation groups (interleaving
                # start=True groups with deferred start=False continuations
                # corrupts PSUM on this toolchain).
                for j in range(4):
                    nc.tensor.matmul(
                        ptv[:, s, j * 64 : (j + 1) * 64],
                        cst[0 : I + 1, C_WIH0 + j * 128 : C_WIH0 + (j + 1) * 128],
                        xt[:, t * BS : (t + 1) * BS],
                        start=True,
                        stop=False,
                    )
                    nc.tensor.matmul(
                        ptv[:, s, j * 64 : (j + 1) * 64],
                        cst[0:H0, C_WHH0 + j * 128 : C_WHH0 + (j + 1) * 128],
                        h0_prev[:],
                        start=False,
                        stop=True,
                    )

            # ---- one bank-wide sigmoid (L0 step t + L1 step t-LAG) ----
            S = spool.tile([128, 512], f32, tag="S")
            nc.scalar.activation(S[:], ptv[:, s, :], AF.Sigmoid, bias=bias0)

            # 3D views: [128, 2 blocks, 64]; block 0 = L0 step t, block 1 = L1 step t-1
            Sv = S[:].rearrange("p (a c) -> p a c", a=2)
            si = Sv[:, :, 0:64]
            sf = Sv[:, :, 64:128]
            so = Sv[:, :, 128:192]
            sg = Sv[:, :, 192:256]

            Pp = wpool.tile([128, 128], f32, tag="Pp")
            Ppv = Pp[:].rearrange("p (a c) -> p a c", a=2)
            v = wpool.tile([128, 128], f32, tag="v")
            vv = v[:].rearrange("p (a c) -> p a c", a=2)
            vc = wpool.tile([128, 128], f32, tag="vc")
            vcv = vc[:].rearrange("p (a c) -> p a c", a=2)
            c_new = wpool.tile([128, 128], f32, tag="c")
            cnv = c_new[:].rearrange("p (a c) -> p a c", a=2)
            cpv = c_prev[:].rearrange("p (a c) -> p a c", a=2)
            Mcv = mpc[:, t * 128 : (t + 1) * 128].rearrange("p (a c) -> p a c", a=2)
            Mhv = mph[:, t * 128 : (t + 1) * 128].rearrange("p (a c) -> p a c", a=2)

            # P' = (sig(g') - 0.5) * sig(i)   [= tanh(g)*sig(i)/2]
            nc.vector.scalar_tensor_tensor(Ppv, sg, 0.5, si, ALU.subtract, ALU.mult)
            # v = sig(f) * c_prev
            nc.vector.tensor_tensor(vv, sf, cpv, ALU.mult)
            # vc = 2*P' + v
            nc.vector.scalar_tensor_tensor(vcv, Ppv, 2.0, vv, ALU.mult, ALU.add)
            # c = (vc * dropout_scale) * mask_c_u8
            nc.vector.scalar_tensor_tensor(cnv, vcv, MASK_SCALE, Mcv, ALU.mult, ALU.mult)
            if t < LAG:
                # layer-1 half of the c state must start at zero (kills psum junk)
                nc.vector.memset(c_new[:, 64:128], 0.0)

            Tt = wpool.tile([128, 128], f32, tag="T")
            # h uses tanh of the UNMASKED cell state (mask only hits the carry)
            nc.scalar.activation(Tt[:], vc[:], AF.Tanh, bias=bias0)

            o2 = wpool.tile([128, 128], f32, tag="o2")
            o2v = o2[:].rearrange("p (a c) -> p a c", a=2)
            # o'' = (sig(o) * dropout_scale) * mask_h_u8
            nc.vector.scalar_tensor_tensor(o2v, so, MASK_SCALE, Mhv, ALU.mult, ALU.mult)

            if t < n_steps:
                h0_new = h0pool.tile([H0, BS], bf16, tag="h0")
                nc.vector.tensor_tensor(h0_new[:], o2[:, 0:64], Tt[:, 0:64], ALU.mult)
                h0_prev2, h0_prev = h0_prev, h0_new
            else:
                h0_prev2 = h0_prev
            if t >= LAG:
                h1_new = hpool.tile([H1 + 1, BS], bf16, tag="h1")
                nc.vector.tensor_tensor(
                    h1_new[0:H1, :], o2[0:H1, 64:128], Tt[0:H1, 64:128], ALU.mult
                )
                h1_prev = h1_new
            c_prev = c_new
            if t in dbg_d:
                dd = dbg_d[t]
                bank_sb = spool.tile([128, 512], f32, tag="bankdbg")
                nc.scalar.copy(bank_sb[:], ptv[:, s, :])
                nc.sync.dma_start(dd["bank"][:], bank_sb[:])
                nc.sync.dma_start(dd["S"][:], S[:])
                nc.sync.dma_start(dd["vc"][:], vc[:])
                nc.sync.dma_start(dd["c"][:], c_new[:])
                nc.sync.dma_start(dd["T"][:], Tt[:])
                nc.sync.dma_start(dd["o2"][:], o2[:])
                nc.sync.dma_start(dd["h0"][:], h0_prev[:])
                nc.sync.dma_start(dd["h1"][:], h1_prev[:])

        # ---- final projection: y = h1_255 @ W_lin.T + b_lin  -> [64, 1] ----
        yp = ptv[0:BS, 1, 0:1]
        nc.tensor.matmul(
            yp, h1_prev[:], cst[0 : H1 + 1, C_WLIN : C_WLIN + 1], start=True, stop=True
        )
        y_sb = const.tile([BS, 1], f32)
        nc.scalar.copy(y_sb[:], yp)
        nc.sync.dma_start(y_d[:], y_sb[:])

    _split_multiwaits(nc)
    return nc


def _split_multiwaits(nc):
    """This toolchain's walrus accepts at most one semaphore wait per
    instruction. Split any extra waits onto standalone EventSemaphore
    instructions inserted just before the offending instruction on the same
    engine queue (in-order execution preserves semantics exactly)."""
    from concourse import mybir

    n = 0
    for fn in nc.m.functions:
        for bb in fn.blocks:
            out = []
            for inst in bb.instructions:
                si = inst.sync_info
                if si is not None and si.on_wait and len(si.on_wait) > 1:
                    waits = list(si.on_wait)
                    for w in waits[:-1]:
                        n += 1
                        out.append(
                            mybir.InstEventSemaphore(
                                name=f"I-wsplit-{n}",
                                engine=inst.engine,
                                ins=[],
                                outs=[],
                                sync_info=mybir.SyncInfo(on_wait=[w], on_update=[]),
                            )
                        )
                    inst.sync_info = mybir.SyncInfo(
                        on_wait=[waits[-1]], on_update=list(si.on_update)
                    )
                out.append(inst)
            bb.instructions = out


def _prep_consts(W_ih0, W_hh0, b_ih0, b_hh0, W_ih1, W_hh1, b_ih1, b_hh1, W_lin, b_lin):
    f = np.float32
    b0 = (b_ih0 + b_hh0).astype(f)
    b1 = (b_ih1 + b_hh1).astype(f)
    cst = np.zeros((128, C_COLS), f)
    for j, g in enumerate(GATE_ORDER):
        m = 2.0 if g == G_GATE else 1.0
        cst[0:I, C_WIH0 + j * H0 : C_WIH0 + (j + 1) * H0] = (
            m * W_ih0[g * H0 : (g + 1) * H0].T
        )
        cst[I, C_WIH0 + j * H0 : C_WIH0 + (j + 1) * H0] = m * b0[g * H0 : (g + 1) * H0]
        cst[0:H0, C_WHH0 + j * H0 : C_WHH0 + (j + 1) * H0] = (
            m * W_hh0[g * H0 : (g + 1) * H0].T
        )
        cst[0:H0, C_WIH1 + j * H1 : C_WIH1 + (j + 1) * H1] = (
            m * W_ih1[g * H1 : (g + 1) * H1].T
        )
        cst[0:H1, C_WHH1 + j * H1 : C_WHH1 + (j + 1) * H1] = (
            m * W_hh1[g * H1 : (g + 1) * H1].T
        )
        cst[H1, C_WHH1 + j * H1 : C_WHH1 + (j + 1) * H1] = m * b1[g * H1 : (g + 1) * H1]
    cst[0:H1, C_WLIN] = W_lin[0]
    cst[H1, C_WLIN] = b_lin[0]
    return cst


def kernel(
    input_seq,
    mask_h0,
    mask_c0,
    mask_h1,
    mask_c1,
    W_ih0,
    W_hh0,
    b_ih0,
    b_hh0,
    W_ih1,
    W_hh1,
    b_ih1,
    b_hh1,
    W_lin,
    b_lin,
):
    import sys

    for p in ("/opt/trn_rl_repo",):
        if p not in sys.path:
            sys.path.insert(0, p)
    from concourse.bass_utils import run_bass_kernel_spmd

    import ml_dtypes

    f = np.float32
    bf = ml_dtypes.bfloat16
    input_seq = np.asarray(input_seq, f)
    mask_h0 = np.asarray(mask_h0, f)
    mask_c0 = np.asarray(mask_c0, f)
    mask_h1 = np.asarray(mask_h1, f)
    mask_c1 = np.asarray(mask_c1, f)
    args = [np.asarray(a, f) for a in (W_ih0, W_hh0, b_ih0, b_hh0,
                                       W_ih1, W_hh1, b_ih1, b_hh1, W_lin, b_lin)]
    cst = _prep_consts(*args).astype(bf)

    in_maps = []
    for c in range(NCORES):
        lo, hi = c * BS, (c + 1) * BS
        xs = input_seq[lo:hi]  # [BS, T, I]
        xt = np.empty((I + 1, T * BS), bf)
        xt[0:I] = xs.transpose(2, 1, 0).reshape(I, T * BS).astype(bf)
        xt[I] = 1.0
        u8 = np.uint8
        LAG = 2
        mph3 = np.zeros((T + LAG, 128, 128), u8)
        mpc3 = np.zeros((T + LAG, 128, 128), u8)
        mph3[0:T, :, 0:64] = (mask_h0[:, lo:hi, :] != 0).transpose(0, 2, 1)
        mph3[LAG : T + LAG, 0:H1, 64:128] = (mask_h1[:, lo:hi, :] != 0).transpose(0, 2, 1)
        mpc3[0:T, :, 0:64] = (mask_c0[:, lo:hi, :] != 0).transpose(0, 2, 1)
        mpc3[LAG : T + LAG, 0:H1, 64:128] = (mask_c1[:, lo:hi, :] != 0).transpose(0, 2, 1)
        mph = np.ascontiguousarray(mph3.transpose(1, 0, 2).reshape(128, -1))
        mpc = np.ascontiguousarray(mpc3.transpose(1, 0, 2).reshape(128, -1))
        in_maps.append({"cst": cst, "xt": xt, "mph": mph, "mpc": mpc})

    if "nc" not in _CACHED:
        _CACHED["nc"] = _build_program()
    _CACHED["in_maps"] = in_maps
    res = run_bass_kernel_spmd(_CACHED["nc"], in_maps, list(range(NCORES)))
    out = np.concatenate([res.results[c]["y"] for c in range(NCORES)], axis=0)
    return out.astype(f)



# revision 16
# speedup vs baseline: 2.1509x; 1.0828x over previous
"""Two-layer dropout-masked LSTM (B=512, T=256, I=64, H0=128, H1=32) on 8 trn2 cores.

Data-parallel over batch: 64 rows/core. State kept transposed [feature, batch].
Per-step PSUM bank layout (512 f32 cols): [i0|f0|o0|g0'|i1|f1|o1|g1'] where the
L1 block holds gates for step t-LAG (L1 runs LAG=2 iterations behind L0, so its
W_ih1 inputs are ready early). All matmul operands are bf16 (PSUM accumulates
f32); L1's weight chunks are zero-padded to 128 output partitions so its
matmuls issue at full rate (narrow 32-partition outputs issue ~3x slower).
tanh(g) = 2*sigmoid(2g) - 1 with the 2x pre-scaled into the g-gate weights, so
ACT runs ONE bank-wide sigmoid + one tanh per step. The whole elementwise chain
is bf16 tensor_tensor/tensor_scalar ops (2x/4x DVE modes; scalar_tensor_tensor
has no fast modes) with the dropout scale baked into bf16 masks.

The toolchain's walrus build supports at most ONE semaphore wait per
instruction, so the program is structured to never need two: static data
(weights/bias/x/masks) arrives via upfront DMAs (masks in 4 column-chunks so
the first steps only wait on chunk 1), an ACT preamble absorbs the const-DMA
tick + loads the sigmoid/tanh table set, a DVE preamble absorbs the mask-chunk
ticks, state inits run on DVE, and a tiny per-group PE dummy matmul absorbs the
PSUM-slot WAR tick.
"""

import numpy as np

B, T, I, H0, H1 = 512, 256, 64, 128, 32
NCORES = 8
BS = B // NCORES  # 64
LAG = 2  # L1 runs LAG iterations behind L0
# col-block order within a PSUM step-bank: i, f, o, g (pytorch rows are i,f,g,o)
GATE_ORDER = [0, 1, 3, 2]
G_GATE = 2  # pytorch block index of the tanh gate, pre-scaled by 2

# packed-constant tensor column layout (L1 chunks padded to 128 wide)
C_WIH0 = 0        # rows 0:65,  cols 0:512
C_WHH0 = 512      # rows 0:128, cols 512:1024
C_WIH1 = 1024     # rows 0:128, cols 1024:1536 (4 chunks x 128, rows 32:128 of each chunk zero)
C_WHH1 = 1536     # rows 0:33,  cols 1536:2048 (ditto)
C_WLIN = 2048     # rows 0:33,  col 2048
C_BIAS = 2049     # zeros col
C_COLS = 2050
MASK_SCALE = float(np.float32(1.0) / np.float32(1.0 - 0.4))
MCHUNK = 4  # mask DMA column-chunks

_CACHED = {}


def _build_program(debug_steps=(), n_steps=T):
    import os
    import concourse.bass as bass
    import concourse.tile as tile
    from concourse import mybir
    from contextlib import ExitStack

    ABL = set(os.environ.get("K_ABLATE", "").split(","))

    f32 = mybir.dt.float32
    bf16 = mybir.dt.bfloat16
    AF = mybir.ActivationFunctionType
    ALU = mybir.AluOpType

    nc = bass.Bass()

    MCOLS = (T + LAG) * 128
    cst_d = nc.declare_dram_parameter("cst", [128, C_COLS], bf16, isOutput=False)
    xt_d = nc.declare_dram_parameter("xt", [I + 1, T * BS], bf16, isOutput=False)
    mph_d = nc.declare_dram_parameter("mph", [128, MCOLS], bf16, isOutput=False)
    mpc_d = nc.declare_dram_parameter("mpc", [128, MCOLS], bf16, isOutput=False)
    y_d = nc.declare_dram_parameter("y", [BS, 1], f32, isOutput=True)
    dbg_d = {}
    for dt_ in debug_steps:
        dbg_d[dt_] = {
            name: nc.declare_dram_parameter(f"dbg_{name}_{dt_}", shape, f32, isOutput=True)
            for name, shape in (
                ("S", [128, 512]), ("vc", [128, 128]), ("c", [128, 128]),
                ("T", [128, 128]), ("o2", [128, 128]), ("h0", [H0, BS]),
                ("h1", [H1 + 1, BS]), ("bank", [128, 512]),
            )
        }

    GRP = 4  # timesteps per PSUM tile (4 banks); bufs=2 -> all 8 banks

    with ExitStack() as ctx:
        tc = ctx.enter_context(tile.TileContext(nc))
        const = ctx.enter_context(tc.tile_pool(name="const", bufs=1))
        xpool = ctx.enter_context(tc.tile_pool(name="xtp", bufs=1))
        psum = ctx.enter_context(
            tc.tile_pool(name="gates", bufs=2, space=bass.MemorySpace.PSUM)
        )
        spool = ctx.enter_context(tc.tile_pool(name="sig", bufs=2))
        mpool = ctx.enter_context(tc.tile_pool(name="masks", bufs=1))
        wpool = ctx.enter_context(tc.tile_pool(name="work", bufs=2))
        hpool = ctx.enter_context(tc.tile_pool(name="state", bufs=2))
        h0pool = ctx.enter_context(tc.tile_pool(name="state0", bufs=3))

        cst = const.tile([128, C_COLS], bf16)
        nc.sync.dma_start(cst[:], cst_d[:])
        xt = xpool.tile([I + 1, T * BS], bf16)
        nc.sync.dma_start(xt[:], xt_d[:])
        # masks arrive in MCHUNK column-chunks: early steps only block on
        # chunk 1 while the rest stream in behind the compute.
        CC = MCOLS // MCHUNK
        mph = mpool.tile([128, MCOLS], bf16, tag="mph")
        mpc = mpool.tile([128, MCOLS], bf16, tag="mpc")
        for ch in range(MCHUNK):
            lo, hi = ch * CC, (ch + 1) * CC
            nc.sync.dma_start(mph[:, lo:hi], mph_d[:, lo:hi])
            nc.sync.dma_start(mpc[:, lo:hi], mpc_d[:, lo:hi])

        bias0 = cst[:, C_BIAS : C_BIAS + 1]

        # ACT preamble: absorb the cst DMA tick on ACT and preload the
        # sigmoid/tanh table set before the timestep loop.
        scratch = const.tile([128, 1], f32)
        nc.scalar.activation(scratch[:], cst[:, 0:1], AF.Copy)
        nc.scalar.activation(scratch[:], cst[:, 0:1], AF.Sigmoid, bias=bias0)
        nc.scalar.activation(scratch[:], cst[:, 0:1], AF.Tanh, bias=bias0)
        # DVE preamble: absorb the mask-chunk DMA ticks so in-loop mask reads
        # never add a second wait on top of same-engine pipeline waits.
        scrm = const.tile([1, 2 * MCHUNK], bf16)
        for ch in range(MCHUNK):
            nc.vector.tensor_copy(scrm[0:1, 2 * ch : 2 * ch + 1], mph[0:1, ch * CC : ch * CC + 1])
            nc.vector.tensor_copy(scrm[0:1, 2 * ch + 1 : 2 * ch + 2], mpc[0:1, ch * CC : ch * CC + 1])

        # ---- initial state (DVE so consumers' waits stay single-source) ----
        h0_prev = h0pool.tile([H0, BS], bf16, tag="h0")
        nc.vector.memset(h0_prev[:], 0.0)
        h0_prev2 = h0_prev
        h1_slot0 = hpool.tile([H1 + 1, BS], bf16, tag="h1")
        nc.vector.memset(h1_slot0[H1 : H1 + 1, :], 1.0)
        h1_prev = hpool.tile([H1 + 1, BS], bf16, tag="h1")
        nc.vector.memset(h1_prev[0:H1, :], 0.0)
        nc.vector.memset(h1_prev[H1 : H1 + 1, :], 1.0)
        c_prev = wpool.tile([128, 128], bf16, tag="c")
        nc.vector.memset(c_prev[:], 0.0)

        pt = None
        ptv = None
        for t in range(n_steps + LAG):
            k, s = divmod(t, GRP)
            if s == 0:
                pt = psum.tile([128, GRP * 512], f32, tag="gates")
                ptv = pt[:].rearrange("p (s c) -> p s c", s=GRP)
                if "dummy" not in ABL:
                    # tiny dummy matmul: absorbs the PSUM-slot WAR (ACT sigma
                    # readers from group k-2) onto PE before any real writer.
                    nc.tensor.matmul(
                        ptv[0:1, GRP - 1, 256:257],
                        cst[0:1, 0:1],
                        cst[0:1, 0:1],
                        start=True,
                        stop=True,
                    )

            if t >= LAG and "l1" not in ABL:
                # L1 gates for step t-LAG (h0_{t-LAG}, h1_{t-LAG-1}; bias via
                # ones row). Emitted BEFORE L0: only the W_hh1 operand is
                # fresh (h1_new from the previous iteration's DVE phase), so
                # these run while L0 still waits on h0_new.
                for j in range(4):
                    nc.tensor.matmul(
                        ptv[:, s, 256 + j * 64 : 256 + (j + 1) * 64],
                        cst[0:H0, C_WIH1 + j * 128 : C_WIH1 + (j + 1) * 128],
                        h0_prev2[:],
                        start=True,
                        stop=False,
                    )
                    nc.tensor.matmul(
                        ptv[:, s, 256 + j * 64 : 256 + (j + 1) * 64],
                        cst[0 : H1 + 1, C_WHH1 + j * 128 : C_WHH1 + (j + 1) * 128],
                        h1_prev[:],
                        start=False,
                        stop=True,
                    )
            if t < n_steps and "rec" not in ABL:
                # L0 gates for step t: x-part (bias via ones row) + recurrent,
                # as immediately-paired accumulation groups (interleaving
                # start=True groups with deferred start=False continuations
                # corrupts PSUM on this toolchain).
                for j in range(4):
                    nc.tensor.matmul(
                        ptv[:, s, j * 64 : (j + 1) * 64],
                        cst[0 : I + 1, C_WIH0 + j * 128 : C_WIH0 + (j + 1) * 128],
                        xt[:, t * BS : (t + 1) * BS],
                        start=True,
                        stop=False,
                    )
                    nc.tensor.matmul(
                        ptv[:, s, j * 64 : (j + 1) * 64],
                        cst[0:H0, C_WHH0 + j * 128 : C_WHH0 + (j + 1) * 128],
                        h0_prev[:],
                        start=False,
                        stop=True,
                    )

            # ---- one bank-wide sigmoid (L0 step t + L1 step t-LAG) ----
            S = spool.tile([128, 512], bf16, tag="S")
            nc.scalar.activation(S[:], ptv[:, s, :], AF.Sigmoid, bias=bias0)

            # 3D views: [128, 2 blocks, 64]; block 0 = L0 step t, block 1 = L1 step t-LAG
            Sv = S[:].rearrange("p (a c) -> p a c", a=2)
            si = Sv[:, :, 0:64]
            sf = Sv[:, :, 64:128]
            so = Sv[:, :, 128:192]
            sg = Sv[:, :, 192:256]

            def wt(tag):
                tl = wpool.tile([128, 128], bf16, tag=tag)
                return tl, tl[:].rearrange("p (a c) -> p a c", a=2)

            tg, tgv = wt("tg")
            P2, P2v = wt("P2")
            v, vv = wt("v")
            vc, vcv = wt("vc")
            c_new, cnv = wt("c")
            o2, o2v = wt("o2")
            cpv = c_prev[:].rearrange("p (a c) -> p a c", a=2)
            Mcv = mpc[:, t * 128 : (t + 1) * 128].rearrange("p (a c) -> p a c", a=2)
            Mhv = mph[:, t * 128 : (t + 1) * 128].rearrange("p (a c) -> p a c", a=2)

            # tanh(g) = 2*sig(g') - 1
            nc.vector.tensor_scalar(tgv, sg, 2.0, -1.0, ALU.mult, ALU.add)
            # P2 = tanh(g) * sig(i)
            nc.vector.tensor_tensor(P2v, tgv, si, ALU.mult)
            # v = sig(f) * c_prev
            nc.vector.tensor_tensor(vv, sf, cpv, ALU.mult)
            # vc = P2 + v  (unmasked cell state)
            nc.vector.tensor_tensor(vcv, P2v, vv, ALU.add)
            # c carry = vc * mask_c (dropout scale baked into the bf16 mask)
            nc.vector.tensor_tensor(cnv, vcv, Mcv, ALU.mult)
            if t < LAG:
                # layer-1 half of the c state must start at zero (kills psum junk)
                nc.vector.memset(c_new[:, 64:128], 0.0)
            # o'' = sig(o) * mask_h (scale baked in)
            nc.vector.tensor_tensor(o2v, so, Mhv, ALU.mult)

            Tt = wpool.tile([128, 128], bf16, tag="T")
            # h uses tanh of the UNMASKED cell state (mask only hits the carry)
            nc.scalar.activation(Tt[:], vc[:], AF.Tanh, bias=bias0)

            # h1 first: next iteration's L1 matmuls wait on it
            if t >= LAG:
                h1_new = hpool.tile([H1 + 1, BS], bf16, tag="h1")
                nc.vector.tensor_tensor(
                    h1_new[0:H1, :], o2[0:H1, 64:128], Tt[0:H1, 64:128], ALU.mult
                )
                h1_prev = h1_new
            if t < n_steps:
                h0_new = h0pool.tile([H0, BS], bf16, tag="h0")
                nc.vector.tensor_tensor(h0_new[:], o2[:, 0:64], Tt[:, 0:64], ALU.mult)
                h0_prev2, h0_prev = h0_prev, h0_new
            else:
                h0_prev2 = h0_prev
            c_prev = c_new
            if t in dbg_d:
                dd = dbg_d[t]
                bank_sb = spool.tile([128, 512], f32, tag="bankdbg")
                nc.scalar.copy(bank_sb[:], ptv[:, s, :])
                nc.sync.dma_start(dd["bank"][:], bank_sb[:])
                for name, tl in (("S", S), ("vc", vc), ("c", c_new),
                                 ("T", Tt), ("o2", o2), ("h0", h0_prev),
                                 ("h1", h1_prev)):
                    f32t = spool.tile(list(tl[:].shape), f32, tag=f"dbg{name}")
                    nc.vector.tensor_copy(f32t[:], tl[:])
                    nc.sync.dma_start(dd[name][:], f32t[:])

        # ---- final projection: y = h1_255 @ W_lin.T + b_lin  -> [64, 1] ----
        yp = ptv[0:BS, 1, 0:1]
        nc.tensor.matmul(
            yp, h1_prev[:], cst[0 : H1 + 1, C_WLIN : C_WLIN + 1], start=True, stop=True
        )
        y_sb = const.tile([BS, 1], f32)
        nc.scalar.copy(y_sb[:], yp)
        nc.sync.dma_start(y_d[:], y_sb[:])

    _split_multiwaits(nc)
    return nc


def _split_multiwaits(nc):
    """This toolchain's walrus accepts at most one semaphore wait per
    instruction. Split any extra waits onto standalone EventSemaphore
    instructions inserted just before the offending instruction on the same
    engine queue (in-order execution preserves semantics exactly)."""
    from concourse import mybir

    n = 0
    for fn in nc.m.functions:
        for bb in fn.blocks:
            out = []
            for inst in bb.instructions:
                si = inst.sync_info
                if si is not None and si.on_wait and len(si.on_wait) > 1:
                    waits = list(si.on_wait)
                    for w in waits[:-1]:
                        n += 1
                        out.append(
                            mybir.InstEventSemaphore(
                                name=f"I-wsplit-{n}",
                                engine=inst.engine,
                                ins=[],
                                outs=[],
                                sync_info=mybir.SyncInfo(on_wait=[w], on_update=[]),
                            )
                        )
                    inst.sync_info = mybir.SyncInfo(
                        on_wait=[waits[-1]], on_update=list(si.on_update)
                    )
                out.append(inst)
            bb.instructions = out


def _prep_consts(W_ih0, W_hh0, b_ih0, b_hh0, W_ih1, W_hh1, b_ih1, b_hh1, W_lin, b_lin):
    f = np.float32
    b0 = (b_ih0 + b_hh0).astype(f)
    b1 = (b_ih1 + b_hh1).astype(f)
    cst = np.zeros((128, C_COLS), f)
    for j, g in enumerate(GATE_ORDER):
        m = 2.0 if g == G_GATE else 1.0
        cst[0:I, C_WIH0 + j * H0 : C_WIH0 + (j + 1) * H0] = (
            m * W_ih0[g * H0 : (g + 1) * H0].T
        )
        cst[I, C_WIH0 + j * H0 : C_WIH0 + (j + 1) * H0] = m * b0[g * H0 : (g + 1) * H0]
        cst[0:H0, C_WHH0 + j * H0 : C_WHH0 + (j + 1) * H0] = (
            m * W_hh0[g * H0 : (g + 1) * H0].T
        )
        # L1 chunks are zero-padded to 128 output partitions (cols j*128..j*128+H1
        # hold the real weights) so the matmuls issue at full rate.
        cst[0:H0, C_WIH1 + j * 128 : C_WIH1 + j * 128 + H1] = (
            m * W_ih1[g * H1 : (g + 1) * H1].T
        )
        cst[0:H1, C_WHH1 + j * 128 : C_WHH1 + j * 128 + H1] = (
            m * W_hh1[g * H1 : (g + 1) * H1].T
        )
        cst[H1, C_WHH1 + j * 128 : C_WHH1 + j * 128 + H1] = m * b1[g * H1 : (g + 1) * H1]
    cst[0:H1, C_WLIN] = W_lin[0]
    cst[H1, C_WLIN] = b_lin[0]
    return cst


def kernel(
    input_seq,
    mask_h0,
    mask_c0,
    mask_h1,
    mask_c1,
    W_ih0,
    W_hh0,
    b_ih0,
    b_hh0,
    W_ih1,
    W_hh1,
    b_ih1,
    b_hh1,
    W_lin,
    b_lin,
):
    import sys

    for p in ("/opt/trn_rl_repo",):
        if p not in sys.path:
            sys.path.insert(0, p)
    from concourse.bass_utils import run_bass_kernel_spmd
    import ml_dtypes

    f = np.float32
    bf = ml_dtypes.bfloat16
    input_seq = np.asarray(input_seq, f)
    mask_h0 = np.asarray(mask_h0, f)
    mask_c0 = np.asarray(mask_c0, f)
    mask_h1 = np.asarray(mask_h1, f)
    mask_c1 = np.asarray(mask_c1, f)
    args = [np.asarray(a, f) for a in (W_ih0, W_hh0, b_ih0, b_hh0,
                                       W_ih1, W_hh1, b_ih1, b_hh1, W_lin, b_lin)]
    cst = _prep_consts(*args).astype(bf)
    scale = np.float32(MASK_SCALE)

    in_maps = []
    for c in range(NCORES):
        lo, hi = c * BS, (c + 1) * BS
        xs = input_seq[lo:hi]  # [BS, T, I]
        xt = np.empty((I + 1, T * BS), bf)
        xt[0:I] = xs.transpose(2, 1, 0).reshape(I, T * BS).astype(bf)
        xt[I] = 1.0
        mph3 = np.zeros((T + LAG, 128, 128), bf)
        mpc3 = np.zeros((T + LAG, 128, 128), bf)
        mph3[0:T, :, 0:64] = (
            ((mask_h0[:, lo:hi, :] != 0) * scale).transpose(0, 2, 1).astype(bf)
        )
        mph3[LAG : T + LAG, 0:H1, 64:128] = (
            ((mask_h1[:, lo:hi, :] != 0) * scale).transpose(0, 2, 1).astype(bf)
        )
        mpc3[0:T, :, 0:64] = (
            ((mask_c0[:, lo:hi, :] != 0) * scale).transpose(0, 2, 1).astype(bf)
        )
        mpc3[LAG : T + LAG, 0:H1, 64:128] = (
            ((mask_c1[:, lo:hi, :] != 0) * scale).transpose(0, 2, 1).astype(bf)
        )
        mph = np.ascontiguousarray(mph3.transpose(1, 0, 2).reshape(128, -1))
        mpc = np.ascontiguousarray(mpc3.transpose(1, 0, 2).reshape(128, -1))
        in_maps.append({"cst": cst, "xt": xt, "mph": mph, "mpc": mpc})

    if "nc" not in _CACHED:
        _CACHED["nc"] = _build_program()
    _CACHED["in_maps"] = in_maps
    res = run_bass_kernel_spmd(_CACHED["nc"], in_maps, list(range(NCORES)))
    out = np.concatenate([res.results[c]["y"] for c in range(NCORES)], axis=0)
    return out.astype(f)


# revision 24
# speedup vs baseline: 2.7136x; 1.2616x over previous
"""Two-layer dropout-masked LSTM (B=512, T=256, I=64, H0=128, H1=32) on 8 trn2 cores.

Data-parallel over batch: 64 rows/core. State kept transposed [feature, batch].
Per-step PSUM bank layout (512 f32 cols): [i0|f0|o0|g0'|i1|f1|o1|g1'] where the
L1 block holds gates for step t-LAG (L1 runs LAG=2 iterations behind L0, so its
W_ih1 inputs are ready early). All matmul operands are bf16 (PSUM accumulates
f32); L1's weight chunks are zero-padded to 128 output partitions so its
matmuls issue at full rate (narrow 32-partition outputs issue ~3x slower).
tanh(g) = 2*sigmoid(2g) - 1 with the 2x pre-scaled into the g-gate weights, so
ACT runs ONE bank-wide sigmoid + one tanh per step. The whole elementwise chain
is bf16 tensor_tensor/tensor_scalar ops (2x/4x DVE modes; scalar_tensor_tensor
has no fast modes) with the dropout scale baked into bf16 masks.

The toolchain's walrus build supports at most ONE semaphore wait per
instruction, so the program is structured to never need two: static data
(weights/bias/x/masks) arrives via upfront DMAs (masks in 4 column-chunks so
the first steps only wait on chunk 1), an ACT preamble absorbs the const-DMA
tick + loads the sigmoid/tanh table set, a DVE preamble absorbs the mask-chunk
ticks, state inits run on DVE, and a tiny per-group PE dummy matmul absorbs the
PSUM-slot WAR tick.
"""

import numpy as np

B, T, I, H0, H1 = 512, 256, 64, 128, 32
NCORES = 8
BS = B // NCORES  # 64
LAG = 2  # L1 runs LAG iterations behind L0
# col-block order within a PSUM step-bank: i, f, g', o (pytorch rows are
# i,f,g,o). Keeping o LAST lets the critical sigmoid cover only [i,f,g'].
GATE_ORDER = [0, 1, 2, 3]
G_GATE = 2  # pytorch block index of the tanh gate, pre-scaled by 2
KPAD = 65  # pad K of the L1 W_hh1 matmuls (and h1 tiles) to 65 rows so every
# matmul in the loop uses the same 128-row PE tile config; K=33 rounds to a
# 64-row tile and the config switch stalls back-to-back matmul issue ~3x.

# packed-constant tensor column layout (L1 chunks padded to 128 wide)
C_WIH0 = 0        # rows 0:65,  cols 0:512
C_WHH0 = 512      # rows 0:128, cols 512:1024
C_WIH1 = 1024     # rows 0:128, cols 1024:1536 (4 chunks x 128, rows 32:128 of each chunk zero)
C_WHH1 = 1536     # rows 0:33,  cols 1536:2048 (ditto)
C_WLIN = 2048     # rows 0:33,  col 2048
C_BIAS = 2049     # zeros col
C_COLS = 2050
MASK_SCALE = float(np.float32(1.0) / np.float32(1.0 - 0.4))
MCHUNK = 4  # mask DMA column-chunks

_CACHED = {}


def _build_program(debug_steps=(), n_steps=T):
    import os
    import concourse.bass as bass
    import concourse.tile as tile
    from concourse import mybir
    from contextlib import ExitStack

    ABL = set(os.environ.get("K_ABLATE", "").split(","))

    f32 = mybir.dt.float32
    bf16 = mybir.dt.bfloat16
    AF = mybir.ActivationFunctionType
    ALU = mybir.AluOpType

    nc = bass.Bass()

    MCOLS = (T + LAG) * 128
    cst_d = nc.declare_dram_parameter("cst", [128, C_COLS], bf16, isOutput=False)
    xt_d = nc.declare_dram_parameter("xt", [I + 1, T * BS], bf16, isOutput=False)
    mph_d = nc.declare_dram_parameter("mph", [128, MCOLS], bf16, isOutput=False)
    mpc_d = nc.declare_dram_parameter("mpc", [128, MCOLS], bf16, isOutput=False)
    y_d = nc.declare_dram_parameter("y", [BS, 1], f32, isOutput=True)
    dbg_d = {}
    for dt_ in debug_steps:
        dbg_d[dt_] = {
            name: nc.declare_dram_parameter(f"dbg_{name}_{dt_}", shape, f32, isOutput=True)
            for name, shape in (
                ("S", [128, 512]), ("vc", [128, 128]), ("c", [128, 128]),
                ("T", [128, 128]), ("o2", [128, 128]), ("h0", [H0, BS]),
                ("h1", [H1 + 1, BS]), ("bank", [128, 512]),
            )
        }

    GRP = 4  # timesteps per PSUM tile (4 banks); bufs=2 -> all 8 banks

    with ExitStack() as ctx:
        tc = ctx.enter_context(tile.TileContext(nc))
        const = ctx.enter_context(tc.tile_pool(name="const", bufs=1))
        xpool = ctx.enter_context(tc.tile_pool(name="xtp", bufs=1))
        psum = ctx.enter_context(
            tc.tile_pool(name="gates", bufs=2, space=bass.MemorySpace.PSUM)
        )
        spool = ctx.enter_context(tc.tile_pool(name="sig", bufs=2))
        mpool = ctx.enter_context(tc.tile_pool(name="masks", bufs=1))
        wpool = ctx.enter_context(tc.tile_pool(name="work", bufs=2))
        hpool = ctx.enter_context(tc.tile_pool(name="state", bufs=2))
        h0pool = ctx.enter_context(tc.tile_pool(name="state0", bufs=3))

        cst = const.tile([128, C_COLS], bf16)
        nc.sync.dma_start(cst[:], cst_d[:])
        xt = xpool.tile([I + 1, T * BS], bf16)
        nc.sync.dma_start(xt[:], xt_d[:])
        # masks arrive in MCHUNK column-chunks: early steps only block on
        # chunk 1 while the rest stream in behind the compute.
        CC = MCOLS // MCHUNK
        mph = mpool.tile([128, MCOLS], bf16, tag="mph")
        mpc = mpool.tile([128, MCOLS], bf16, tag="mpc")
        for ch in range(MCHUNK):
            lo, hi = ch * CC, (ch + 1) * CC
            nc.sync.dma_start(mph[:, lo:hi], mph_d[:, lo:hi])
            nc.sync.dma_start(mpc[:, lo:hi], mpc_d[:, lo:hi])

        bias0 = cst[:, C_BIAS : C_BIAS + 1]

        # ACT preamble: absorb the cst DMA tick on ACT and preload the
        # sigmoid/tanh table set before the timestep loop.
        scratch = const.tile([128, 1], f32)
        nc.scalar.activation(scratch[:], cst[:, 0:1], AF.Copy)
        nc.scalar.activation(scratch[:], cst[:, 0:1], AF.Sigmoid, bias=bias0)
        nc.scalar.activation(scratch[:], cst[:, 0:1], AF.Tanh, bias=bias0)
        # DVE preamble: absorb the mask-chunk DMA ticks so in-loop mask reads
        # never add a second wait on top of same-engine pipeline waits.
        scrm = const.tile([1, 2 * MCHUNK], bf16)
        for ch in range(MCHUNK):
            nc.vector.tensor_copy(scrm[0:1, 2 * ch : 2 * ch + 1], mph[0:1, ch * CC : ch * CC + 1])
            nc.vector.tensor_copy(scrm[0:1, 2 * ch + 1 : 2 * ch + 2], mpc[0:1, ch * CC : ch * CC + 1])

        # ---- initial state (DVE so consumers' waits stay single-source) ----
        h0_prev = h0pool.tile([H0, BS], bf16, tag="h0")
        nc.vector.memset(h0_prev[:], 0.0)
        h0_prev2 = h0_prev
        # h1 tiles are [KPAD, BS]: row 32 = ones (bias row), rows 33:KPAD = 0
        # (multiplied by zero weight rows; zeroed once so no NaN*0). Init both
        # ring buffers.
        # (memset APs not starting at partition 0 may span at most 32
        # partitions, so the 32:65 range is zeroed as 32:64 + 64:65)
        h1_slot0 = hpool.tile([KPAD, BS], bf16, tag="h1")
        nc.vector.memset(h1_slot0[32:64, :], 0.0)
        nc.vector.memset(h1_slot0[64:KPAD, :], 0.0)
        nc.vector.memset(h1_slot0[H1 : H1 + 1, :], 1.0)
        h1_prev = hpool.tile([KPAD, BS], bf16, tag="h1")
        nc.vector.memset(h1_prev[0:H1, :], 0.0)
        nc.vector.memset(h1_prev[32:64, :], 0.0)
        nc.vector.memset(h1_prev[64:KPAD, :], 0.0)
        nc.vector.memset(h1_prev[H1 : H1 + 1, :], 1.0)
        c_prev = wpool.tile([128, 128], bf16, tag="c")
        nc.vector.memset(c_prev[:], 0.0)

        pt = None
        ptv = None
        for t in range(n_steps + LAG):
            k, s = divmod(t, GRP)
            if s == 0:
                pt = psum.tile([128, GRP * 512], f32, tag="gates")
                ptv = pt[:].rearrange("p (s c) -> p s c", s=GRP)
                if "dummy" not in ABL:
                    # dummy matmul: absorbs the PSUM-slot WAR (ACT sigma
                    # readers from group k-2) onto PE before any real writer.
                    # Shaped K=65/M=64/N=64 to match the loop's PE tile config
                    # (a K=1 matmul would force a 32-row tile switch).
                    nc.tensor.matmul(
                        ptv[0:64, GRP - 1, 256:320],
                        cst[0:KPAD, 0:64],
                        cst[0:KPAD, 0:64],
                        start=True,
                        stop=True,
                    )

            if t >= LAG and "l1" not in ABL:
                # L1 gates for step t-LAG (h0_{t-LAG}, h1_{t-LAG-1}; bias via
                # ones row). Emitted BEFORE L0: only the W_hh1 operand is
                # fresh (h1_new from the previous iteration's DVE phase), so
                # these run while L0 still waits on h0_new.
                for j in range(4):
                    nc.tensor.matmul(
                        ptv[:, s, 256 + j * 64 : 256 + (j + 1) * 64],
                        cst[0:H0, C_WIH1 + j * 128 : C_WIH1 + (j + 1) * 128],
                        h0_prev2[:],
                        start=True,
                        stop=False,
                    )
                    nc.tensor.matmul(
                        ptv[:, s, 256 + j * 64 : 256 + (j + 1) * 64],
                        cst[0:KPAD, C_WHH1 + j * 128 : C_WHH1 + (j + 1) * 128],
                        h1_prev[:],
                        start=False,
                        stop=True,
                    )
            if t < n_steps and "rec" not in ABL:
                # L0 gates for step t: x-part (bias via ones row) + recurrent,
                # as immediately-paired accumulation groups (interleaving
                # start=True groups with deferred start=False continuations
                # corrupts PSUM on this toolchain).
                for j in range(4):
                    nc.tensor.matmul(
                        ptv[:, s, j * 64 : (j + 1) * 64],
                        cst[0 : I + 1, C_WIH0 + j * 128 : C_WIH0 + (j + 1) * 128],
                        xt[:, t * BS : (t + 1) * BS],
                        start=True,
                        stop=False,
                    )
                    nc.tensor.matmul(
                        ptv[:, s, j * 64 : (j + 1) * 64],
                        cst[0:H0, C_WHH0 + j * 128 : C_WHH0 + (j + 1) * 128],
                        h0_prev[:],
                        start=False,
                        stop=True,
                    )

            # ---- sigmoids: critical [i,f,g'] cols first, o-gate cols in a
            # second instruction that runs under the DVE chain ----
            S = spool.tile([128, 512], bf16, tag="S")
            Sv = S[:].rearrange("p (a c) -> p a c", a=2)
            bankv = ptv[:, s, :].rearrange("p (a c) -> p a c", a=2)
            nc.scalar.activation(Sv[:, :, 0:192], bankv[:, :, 0:192], AF.Sigmoid, bias=bias0)
            nc.scalar.activation(Sv[:, :, 192:256], bankv[:, :, 192:256], AF.Sigmoid, bias=bias0)

            # block 0 = L0 step t, block 1 = L1 step t-LAG
            si = Sv[:, :, 0:64]
            sf = Sv[:, :, 64:128]
            sg = Sv[:, :, 128:192]
            so = Sv[:, :, 192:256]

            def wt(tag):
                tl = wpool.tile([128, 128], bf16, tag=tag)
                return tl, tl[:].rearrange("p (a c) -> p a c", a=2)

            Pp, Ppv = wt("Pp")
            v, vv = wt("v")
            vc, vcv = wt("vc")
            c_new, cnv = wt("c")
            o2, o2v = wt("o2")
            cpv = c_prev[:].rearrange("p (a c) -> p a c", a=2)
            Mcv = mpc[:, t * 128 : (t + 1) * 128].rearrange("p (a c) -> p a c", a=2)
            Mhv = mph[:, t * 128 : (t + 1) * 128].rearrange("p (a c) -> p a c", a=2)

            # The cell state is carried as cc = c/2 ("half-cell"): the 2x is
            # recovered inside tanh via its scale operand, so the chain after
            # the sigmoid is only 3 ops deep.
            # Pp = (sig(g') - 0.5) * sig(i)  [= tanh(g)*sig(i)/2]
            nc.vector.scalar_tensor_tensor(Ppv, sg, 0.5, si, ALU.subtract, ALU.mult)
            # v = sig(f) * cc_prev
            nc.vector.tensor_tensor(vv, sf, cpv, ALU.mult)
            # vc = Pp + v  (unmasked half cell state)
            nc.vector.tensor_tensor(vcv, Ppv, vv, ALU.add)
            # cc carry = vc * mask_c (dropout scale baked into the bf16 mask)
            nc.vector.tensor_tensor(cnv, vcv, Mcv, ALU.mult)
            if t < LAG:
                # layer-1 half of the c state must start at zero (kills psum junk)
                nc.vector.memset(c_new[:, 64:128], 0.0)
            # o'' = sig(o) * mask_h (scale baked in)
            nc.vector.tensor_tensor(o2v, so, Mhv, ALU.mult)

            Tt = wpool.tile([128, 128], bf16, tag="T")
            # h uses tanh of the UNMASKED cell state (mask only hits the
            # carry); scale=2 recovers c from the half-cell representation
            nc.scalar.activation(Tt[:], vc[:], AF.Tanh, bias=bias0, scale=2.0)

            # h1 first: next iteration's L1 matmuls wait on it
            if t >= LAG:
                h1_new = hpool.tile([KPAD, BS], bf16, tag="h1")
                nc.vector.tensor_tensor(
                    h1_new[0:H1, :], o2[0:H1, 64:128], Tt[0:H1, 64:128], ALU.mult
                )
                h1_prev = h1_new
            if t < n_steps:
                h0_new = h0pool.tile([H0, BS], bf16, tag="h0")
                nc.vector.tensor_tensor(h0_new[:], o2[:, 0:64], Tt[:, 0:64], ALU.mult)
                h0_prev2, h0_prev = h0_prev, h0_new
            else:
                h0_prev2 = h0_prev
            c_prev = c_new
            if t in dbg_d:
                dd = dbg_d[t]
                bank_sb = spool.tile([128, 512], f32, tag="bankdbg")
                nc.scalar.copy(bank_sb[:], ptv[:, s, :])
                nc.sync.dma_start(dd["bank"][:], bank_sb[:])
                for name, tl in (("S", S), ("vc", vc), ("c", c_new),
                                 ("T", Tt), ("o2", o2), ("h0", h0_prev),
                                 ("h1", h1_prev)):
                    f32t = spool.tile(list(tl[:].shape), f32, tag=f"dbg{name}")
                    nc.vector.tensor_copy(f32t[:], tl[:])
                    nc.sync.dma_start(dd[name][:], f32t[:])

        # ---- final projection: y = h1_255 @ W_lin.T + b_lin  -> [64, 1] ----
        yp = ptv[0:BS, 1, 0:1]
        nc.tensor.matmul(
            yp, h1_prev[:], cst[0:KPAD, C_WLIN : C_WLIN + 1], start=True, stop=True
        )
        y_sb = const.tile([BS, 1], f32)
        nc.scalar.copy(y_sb[:], yp)
        nc.sync.dma_start(y_d[:], y_sb[:])

    _split_multiwaits(nc)
    return nc


def _split_multiwaits(nc):
    """This toolchain's walrus accepts at most one semaphore wait per
    instruction. Split any extra waits onto standalone EventSemaphore
    instructions inserted just before the offending instruction on the same
    engine queue (in-order execution preserves semantics exactly)."""
    from concourse import mybir

    n = 0
    for fn in nc.m.functions:
        for bb in fn.blocks:
            out = []
            for inst in bb.instructions:
                si = inst.sync_info
                if si is not None and si.on_wait and len(si.on_wait) > 1:
                    waits = list(si.on_wait)
                    for w in waits[:-1]:
                        n += 1
                        out.append(
                            mybir.InstEventSemaphore(
                                name=f"I-wsplit-{n}",
                                engine=inst.engine,
                                ins=[],
                                outs=[],
                                sync_info=mybir.SyncInfo(on_wait=[w], on_update=[]),
                            )
                        )
                    inst.sync_info = mybir.SyncInfo(
                        on_wait=[waits[-1]], on_update=list(si.on_update)
                    )
                out.append(inst)
            bb.instructions = out


def _prep_consts(W_ih0, W_hh0, b_ih0, b_hh0, W_ih1, W_hh1, b_ih1, b_hh1, W_lin, b_lin):
    f = np.float32
    b0 = (b_ih0 + b_hh0).astype(f)
    b1 = (b_ih1 + b_hh1).astype(f)
    cst = np.zeros((128, C_COLS), f)
    for j, g in enumerate(GATE_ORDER):
        m = 2.0 if g == G_GATE else 1.0
        cst[0:I, C_WIH0 + j * H0 : C_WIH0 + (j + 1) * H0] = (
            m * W_ih0[g * H0 : (g + 1) * H0].T
        )
        cst[I, C_WIH0 + j * H0 : C_WIH0 + (j + 1) * H0] = m * b0[g * H0 : (g + 1) * H0]
        cst[0:H0, C_WHH0 + j * H0 : C_WHH0 + (j + 1) * H0] = (
            m * W_hh0[g * H0 : (g + 1) * H0].T
        )
        # L1 chunks are zero-padded to 128 output partitions (cols j*128..j*128+H1
        # hold the real weights) so the matmuls issue at full rate.
        cst[0:H0, C_WIH1 + j * 128 : C_WIH1 + j * 128 + H1] = (
            m * W_ih1[g * H1 : (g + 1) * H1].T
        )
        cst[0:H1, C_WHH1 + j * 128 : C_WHH1 + j * 128 + H1] = (
            m * W_hh1[g * H1 : (g + 1) * H1].T
        )
        cst[H1, C_WHH1 + j * 128 : C_WHH1 + j * 128 + H1] = m * b1[g * H1 : (g + 1) * H1]
    cst[0:H1, C_WLIN] = W_lin[0]
    cst[H1, C_WLIN] = b_lin[0]
    return cst


def kernel(
    input_seq,
    mask_h0,
    mask_c0,
    mask_h1,
    mask_c1,
    W_ih0,
    W_hh0,
    b_ih0,
    b_hh0,
    W_ih1,
    W_hh1,
    b_ih1,
    b_hh1,
    W_lin,
    b_lin,
):
    import sys

    for p in ("/opt/trn_rl_repo",):
        if p not in sys.path:
            sys.path.insert(0, p)
    from concourse.bass_utils import run_bass_kernel_spmd
    import ml_dtypes

    f = np.float32
    bf = ml_dtypes.bfloat16
    input_seq = np.asarray(input_seq, f)
    mask_h0 = np.asarray(mask_h0, f)
    mask_c0 = np.asarray(mask_c0, f)
    mask_h1 = np.asarray(mask_h1, f)
    mask_c1 = np.asarray(mask_c1, f)
    args = [np.asarray(a, f) for a in (W_ih0, W_hh0, b_ih0, b_hh0,
                                       W_ih1, W_hh1, b_ih1, b_hh1, W_lin, b_lin)]
    cst = _prep_consts(*args).astype(bf)
    scale = np.float32(MASK_SCALE)

    in_maps = []
    for c in range(NCORES):
        lo, hi = c * BS, (c + 1) * BS
        xs = input_seq[lo:hi]  # [BS, T, I]
        xt = np.empty((I + 1, T * BS), bf)
        xt[0:I] = xs.transpose(2, 1, 0).reshape(I, T * BS).astype(bf)
        xt[I] = 1.0
        mph3 = np.zeros((T + LAG, 128, 128), bf)
        mpc3 = np.zeros((T + LAG, 128, 128), bf)
        mph3[0:T, :, 0:64] = (
            ((mask_h0[:, lo:hi, :] != 0) * scale).transpose(0, 2, 1).astype(bf)
        )
        mph3[LAG : T + LAG, 0:H1, 64:128] = (
            ((mask_h1[:, lo:hi, :] != 0) * scale).transpose(0, 2, 1).astype(bf)
        )
        mpc3[0:T, :, 0:64] = (
            ((mask_c0[:, lo:hi, :] != 0) * scale).transpose(0, 2, 1).astype(bf)
        )
        mpc3[LAG : T + LAG, 0:H1, 64:128] = (
            ((mask_c1[:, lo:hi, :] != 0) * scale).transpose(0, 2, 1).astype(bf)
        )
        mph = np.ascontiguousarray(mph3.transpose(1, 0, 2).reshape(128, -1))
        mpc = np.ascontiguousarray(mpc3.transpose(1, 0, 2).reshape(128, -1))
        in_maps.append({"cst": cst, "xt": xt, "mph": mph, "mpc": mpc})

    if "nc" not in _CACHED:
        _CACHED["nc"] = _build_program()
    _CACHED["in_maps"] = in_maps
    res = run_bass_kernel_spmd(_CACHED["nc"], in_maps, list(range(NCORES)))
    out = np.concatenate([res.results[c]["y"] for c in range(NCORES)], axis=0)
    return out.astype(f)


# revision 32
# speedup vs baseline: 2.8171x; 1.0381x over previous
"""Two-layer dropout-masked LSTM (B=512, T=256, I=64, H0=128, H1=32) on 8 trn2 cores.

Data-parallel over batch: 64 rows/core. State kept transposed [feature, batch].
Per-step PSUM bank layout (512 f32 cols): [i0|f0|o0|g0'|i1|f1|o1|g1'] where the
L1 block holds gates for step t-LAG (L1 runs LAG=2 iterations behind L0, so its
W_ih1 inputs are ready early). All matmul operands are bf16 (PSUM accumulates
f32); L1's weight chunks are zero-padded to 128 output partitions so its
matmuls issue at full rate (narrow 32-partition outputs issue ~3x slower).
tanh(g) = 2*sigmoid(2g) - 1 with the 2x pre-scaled into the g-gate weights, so
ACT runs ONE bank-wide sigmoid + one tanh per step. The whole elementwise chain
is bf16 tensor_tensor/tensor_scalar ops (2x/4x DVE modes; scalar_tensor_tensor
has no fast modes) with the dropout scale baked into bf16 masks.

The toolchain's walrus build supports at most ONE semaphore wait per
instruction, so the program is structured to never need two: static data
(weights/bias/x/masks) arrives via upfront DMAs (masks in 4 column-chunks so
the first steps only wait on chunk 1), an ACT preamble absorbs the const-DMA
tick + loads the sigmoid/tanh table set, a DVE preamble absorbs the mask-chunk
ticks, state inits run on DVE, and a tiny per-group PE dummy matmul absorbs the
PSUM-slot WAR tick.
"""

import numpy as np

B, T, I, H0, H1 = 512, 256, 64, 128, 32
NCORES = 8
BS = B // NCORES  # 64
LAG = 2  # L1 runs LAG iterations behind L0
# col-block order within a PSUM step-bank: i, f, g', o (pytorch rows are
# i,f,g,o). Keeping o LAST lets the critical sigmoid cover only [i,f,g'].
GATE_ORDER = [0, 1, 2, 3]
G_GATE = 2  # pytorch block index of the tanh gate, pre-scaled by 2
KPAD = 65  # pad K of the L1 W_hh1 matmuls (and h1 tiles) to 65 rows so every
# matmul in the loop uses the same 128-row PE tile config; K=33 rounds to a
# 64-row tile and the config switch stalls back-to-back matmul issue ~3x.

# packed-constant tensor column layout (L1 chunks padded to 128 wide)
C_WIH0 = 0        # rows 0:65,  cols 0:512
C_WHH0 = 512      # rows 0:128, cols 512:1024
C_WIH1 = 1024     # rows 0:128, cols 1024:1536 (4 chunks x 128, rows 32:128 of each chunk zero)
C_WHH1 = 1536     # rows 0:33,  cols 1536:2048 (ditto)
C_WLIN = 2048     # rows 0:33,  col 2048
C_BIAS = 2049     # zeros col
C_COLS = 2050
MASK_SCALE = float(np.float32(1.0) / np.float32(1.0 - 0.4))
MCHUNK = 4  # mask DMA column-chunks

_CACHED = {}


def _build_program(debug_steps=(), n_steps=T):
    import os
    import concourse.bass as bass
    import concourse.tile as tile
    from concourse import mybir
    from contextlib import ExitStack

    ABL = set(os.environ.get("K_ABLATE", "").split(","))

    f32 = mybir.dt.float32
    bf16 = mybir.dt.bfloat16
    AF = mybir.ActivationFunctionType
    ALU = mybir.AluOpType

    nc = bass.Bass()

    MCOLS = (T + LAG) * 128
    cst_d = nc.declare_dram_parameter("cst", [128, C_COLS], bf16, isOutput=False)
    xt_d = nc.declare_dram_parameter("xt", [I + 1, T * BS], bf16, isOutput=False)
    mph_d = nc.declare_dram_parameter("mph", [128, MCOLS], bf16, isOutput=False)
    mpc_d = nc.declare_dram_parameter("mpc", [128, MCOLS], bf16, isOutput=False)
    y_d = nc.declare_dram_parameter("y", [BS, 1], f32, isOutput=True)
    dbg_d = {}
    for dt_ in debug_steps:
        dbg_d[dt_] = {
            name: nc.declare_dram_parameter(f"dbg_{name}_{dt_}", shape, f32, isOutput=True)
            for name, shape in (
                ("S", [128, 512]), ("vc", [128, 128]), ("c", [128, 128]),
                ("T", [128, 128]), ("o2", [128, 128]), ("h0", [H0, BS]),
                ("h1", [H1 + 1, BS]), ("bank", [128, 512]),
            )
        }

    GRP = 3  # timesteps per PSUM tile (3 banks); bufs=2 -> 6 banks, leaving
    # room for the never-read p-state filler bank.
    NFILL = int(os.environ.get("K_FILL", "8"))  # p-state filler matmuls/step
    FCOLS = 256

    with ExitStack() as ctx:
        tc = ctx.enter_context(tile.TileContext(nc))
        const = ctx.enter_context(tc.tile_pool(name="const", bufs=1))
        xpool = ctx.enter_context(tc.tile_pool(name="xtp", bufs=1))
        psum = ctx.enter_context(
            tc.tile_pool(name="gates", bufs=2, space=bass.MemorySpace.PSUM)
        )
        spool = ctx.enter_context(tc.tile_pool(name="sig", bufs=2))
        mpool = ctx.enter_context(tc.tile_pool(name="masks", bufs=1))
        wpool = ctx.enter_context(tc.tile_pool(name="work", bufs=2))
        hpool = ctx.enter_context(tc.tile_pool(name="state", bufs=2))
        h0pool = ctx.enter_context(tc.tile_pool(name="state0", bufs=3))
        fpool = ctx.enter_context(
            tc.tile_pool(name="fill", bufs=1, space=bass.MemorySpace.PSUM)
        )

        cst = const.tile([128, C_COLS], bf16)
        nc.sync.dma_start(cst[:], cst_d[:])
        xt = xpool.tile([I + 1, T * BS], bf16)
        nc.sync.dma_start(xt[:], xt_d[:])
        # masks arrive in MCHUNK column-chunks: early steps only block on
        # chunk 1 while the rest stream in behind the compute.
        CC = MCOLS // MCHUNK
        mph = mpool.tile([128, MCOLS], bf16, tag="mph")
        mpc = mpool.tile([128, MCOLS], bf16, tag="mpc")
        for ch in range(MCHUNK):
            lo, hi = ch * CC, (ch + 1) * CC
            nc.sync.dma_start(mph[:, lo:hi], mph_d[:, lo:hi])
            nc.sync.dma_start(mpc[:, lo:hi], mpc_d[:, lo:hi])

        bias0 = cst[:, C_BIAS : C_BIAS + 1]

        # ACT preamble: absorb the cst DMA tick on ACT and preload the
        # sigmoid/tanh table set before the timestep loop.
        scratch = const.tile([128, 1], f32)
        nc.scalar.activation(scratch[:], cst[:, 0:1], AF.Copy)
        nc.scalar.activation(scratch[:], cst[:, 0:1], AF.Sigmoid, bias=bias0)
        nc.scalar.activation(scratch[:], cst[:, 0:1], AF.Tanh, bias=bias0)
        # DVE preamble: absorb the mask-chunk DMA ticks so in-loop mask reads
        # never add a second wait on top of same-engine pipeline waits.
        scrm = const.tile([1, 2 * MCHUNK], bf16)
        for ch in range(MCHUNK):
            nc.vector.tensor_copy(scrm[0:1, 2 * ch : 2 * ch + 1], mph[0:1, ch * CC : ch * CC + 1])
            nc.vector.tensor_copy(scrm[0:1, 2 * ch + 1 : 2 * ch + 2], mpc[0:1, ch * CC : ch * CC + 1])

        # ---- initial state (DVE so consumers' waits stay single-source) ----
        h0_prev = h0pool.tile([H0, BS], bf16, tag="h0")
        nc.vector.memset(h0_prev[:], 0.0)
        h0_prev2 = h0_prev
        # h1 tiles are [KPAD, BS]: row 32 = ones (bias row), rows 33:KPAD = 0
        # (multiplied by zero weight rows; zeroed once so no NaN*0). Init both
        # ring buffers.
        # (memset APs not starting at partition 0 may span at most 32
        # partitions, so the 32:65 range is zeroed as 32:64 + 64:65)
        h1_slot0 = hpool.tile([KPAD, BS], bf16, tag="h1")
        nc.vector.memset(h1_slot0[32:64, :], 0.0)
        nc.vector.memset(h1_slot0[64:KPAD, :], 0.0)
        nc.vector.memset(h1_slot0[H1 : H1 + 1, :], 1.0)
        h1_prev = hpool.tile([KPAD, BS], bf16, tag="h1")
        nc.vector.memset(h1_prev[0:H1, :], 0.0)
        nc.vector.memset(h1_prev[32:64, :], 0.0)
        nc.vector.memset(h1_prev[64:KPAD, :], 0.0)
        nc.vector.memset(h1_prev[H1 : H1 + 1, :], 1.0)
        c_prev = wpool.tile([128, 128], bf16, tag="c")
        nc.vector.memset(c_prev[:], 0.0)
        fill = None
        if NFILL:
            fill = fpool.tile([128, 512], f32, tag="fill")

        pt = None
        ptv = None
        for t in range(n_steps + LAG):
            k, s = divmod(t, GRP)
            if s == 0:
                pt = psum.tile([128, GRP * 512], f32, tag="gates")
                ptv = pt[:].rearrange("p (s c) -> p s c", s=GRP)
                if "dummy" not in ABL:
                    # dummy matmul: absorbs the PSUM-slot WAR (ACT sigma
                    # readers from group k-2) onto PE before any real writer.
                    # Shaped K=65/M=128/N=64 to match the loop's PE tile
                    # config (any other shape forces a tile-config switch).
                    nc.tensor.matmul(
                        ptv[:, GRP - 1, 256:320],
                        cst[0:KPAD, 0:128],
                        cst[0:KPAD, 0:64],
                        start=True,
                        stop=True,
                    )

            if t < n_steps and "rec" not in ABL:
                # L0 gates for step t: x-part (bias via ones row) + recurrent,
                # as immediately-paired accumulation groups (interleaving
                # start=True groups with deferred start=False continuations
                # corrupts PSUM on this toolchain).
                for j in range(4):
                    nc.tensor.matmul(
                        ptv[:, s, j * 64 : (j + 1) * 64],
                        cst[0 : I + 1, C_WIH0 + j * 128 : C_WIH0 + (j + 1) * 128],
                        xt[:, t * BS : (t + 1) * BS],
                        start=True,
                        stop=False,
                    )
                    nc.tensor.matmul(
                        ptv[:, s, j * 64 : (j + 1) * 64],
                        cst[0:H0, C_WHH0 + j * 128 : C_WHH0 + (j + 1) * 128],
                        h0_prev[:],
                        start=False,
                        stop=True,
                    )
            if t >= LAG and "l1" not in ABL:
                # L1 gates for step t-LAG (h0_{t-LAG}, h1_{t-LAG-1}; bias via
                # ones row). Emitted after L0: h1_new (tanh_L1-gated) lands
                # later than h0_new, so these stream behind L0's block.
                for j in range(4):
                    nc.tensor.matmul(
                        ptv[:, s, 256 + j * 64 : 256 + (j + 1) * 64],
                        cst[0:H0, C_WIH1 + j * 128 : C_WIH1 + (j + 1) * 128],
                        h0_prev2[:],
                        start=True,
                        stop=False,
                    )
                    nc.tensor.matmul(
                        ptv[:, s, 256 + j * 64 : 256 + (j + 1) * 64],
                        cst[0:KPAD, C_WHH1 + j * 128 : C_WHH1 + (j + 1) * 128],
                        h1_prev[:],
                        start=False,
                        stop=True,
                    )
            if NFILL and "fill" not in ABL:
                # p-state fillers: never-read matmuls that keep the PE clock
                # ramped while ACT/DVE run the recurrence chain. Tile config
                # matches the loop's so they stream at full rate.
                for _ in range(NFILL):
                    nc.tensor.matmul(
                        fill[:, 0:FCOLS],
                        cst[:, 0:128],
                        cst[:, 0:FCOLS],
                        start=True,
                        stop=True,
                    )

            # ---- sigmoids: critical [i,f,g'] cols first, o-gate cols in a
            # second instruction that runs under the DVE chain ----
            S = spool.tile([128, 512], bf16, tag="S")
            Sv = S[:].rearrange("p (a c) -> p a c", a=2)
            bankv = ptv[:, s, :].rearrange("p (a c) -> p a c", a=2)
            nc.scalar.activation(Sv[:, :, 0:192], bankv[:, :, 0:192], AF.Sigmoid, bias=bias0)
            nc.scalar.activation(Sv[:, :, 192:256], bankv[:, :, 192:256], AF.Sigmoid, bias=bias0)

            # block 0 = L0 step t, block 1 = L1 step t-LAG
            si = Sv[:, :, 0:64]
            sf = Sv[:, :, 64:128]
            sg = Sv[:, :, 128:192]
            so = Sv[:, :, 192:256]

            def wt(tag):
                tl = wpool.tile([128, 128], bf16, tag=tag)
                return tl, tl[:].rearrange("p (a c) -> p a c", a=2)

            Pp, Ppv = wt("Pp")
            v, vv = wt("v")
            vc, vcv = wt("vc")
            c_new, cnv = wt("c")
            o2, o2v = wt("o2")
            cpv = c_prev[:].rearrange("p (a c) -> p a c", a=2)
            Mcv = mpc[:, t * 128 : (t + 1) * 128].rearrange("p (a c) -> p a c", a=2)
            Mhv = mph[:, t * 128 : (t + 1) * 128].rearrange("p (a c) -> p a c", a=2)

            # The cell state is carried as cc = c/2 ("half-cell"): the 2x is
            # recovered inside tanh via its scale operand, so the chain after
            # the sigmoid is only 3 ops deep.
            # Pp = (sig(g') - 0.5) * sig(i)  [= tanh(g)*sig(i)/2]
            nc.vector.scalar_tensor_tensor(Ppv, sg, 0.5, si, ALU.subtract, ALU.mult)
            # v = sig(f) * cc_prev
            nc.vector.tensor_tensor(vv, sf, cpv, ALU.mult)
            # vc = Pp + v  (unmasked half cell state)
            nc.vector.tensor_tensor(vcv, Ppv, vv, ALU.add)
            # o'' = sig(o) * mask_h (scale baked in)
            nc.vector.tensor_tensor(o2v, so, Mhv, ALU.mult)

            Tt = wpool.tile([128, 128], bf16, tag="T")
            # h uses tanh of the UNMASKED cell state (mask only hits the
            # carry); scale=2 recovers c from the half-cell representation.
            # Split per layer: h0_new only needs the L0 half, so L0's matmul
            # block starts before the L1 half is even computed.
            nc.scalar.activation(Tt[:, 0:64], vc[:, 0:64], AF.Tanh, bias=bias0, scale=2.0)
            nc.scalar.activation(Tt[:, 64:128], vc[:, 64:128], AF.Tanh, bias=bias0, scale=2.0)

            if t < n_steps:
                h0_new = h0pool.tile([H0, BS], bf16, tag="h0")
                nc.vector.tensor_tensor(h0_new[:], o2[:, 0:64], Tt[:, 0:64], ALU.mult)
                h0_prev2, h0_prev = h0_prev, h0_new
            else:
                h0_prev2 = h0_prev
            if t >= LAG:
                h1_new = hpool.tile([KPAD, BS], bf16, tag="h1")
                nc.vector.tensor_tensor(
                    h1_new[0:H1, :], o2[0:H1, 64:128], Tt[0:H1, 64:128], ALU.mult
                )
                h1_prev = h1_new
            # cc carry = vc * mask_c (dropout scale baked into the bf16 mask);
            # only needed by the NEXT step's v, so it runs after the h ops.
            nc.vector.tensor_tensor(cnv, vcv, Mcv, ALU.mult)
            if t < LAG:
                # layer-1 half of the c state must start at zero (kills psum junk)
                nc.vector.memset(c_new[:, 64:128], 0.0)
            c_prev = c_new
            if t in dbg_d:
                dd = dbg_d[t]
                bank_sb = spool.tile([128, 512], f32, tag="bankdbg")
                nc.scalar.copy(bank_sb[:], ptv[:, s, :])
                nc.sync.dma_start(dd["bank"][:], bank_sb[:])
                for name, tl in (("S", S), ("vc", vc), ("c", c_new),
                                 ("T", Tt), ("o2", o2), ("h0", h0_prev),
                                 ("h1", h1_prev)):
                    f32t = spool.tile(list(tl[:].shape), f32, tag=f"dbg{name}")
                    nc.vector.tensor_copy(f32t[:], tl[:])
                    nc.sync.dma_start(dd[name][:], f32t[:])

        # ---- final projection: y = h1_255 @ W_lin.T + b_lin  -> [64, 1] ----
        yp = ptv[0:BS, 1, 0:1]
        nc.tensor.matmul(
            yp, h1_prev[:], cst[0:KPAD, C_WLIN : C_WLIN + 1], start=True, stop=True
        )
        y_sb = const.tile([BS, 1], f32)
        nc.scalar.copy(y_sb[:], yp)
        nc.sync.dma_start(y_d[:], y_sb[:])

    _split_multiwaits(nc)
    return nc


def _split_multiwaits(nc):
    """This toolchain's walrus accepts at most one semaphore wait per
    instruction. Split any extra waits onto standalone EventSemaphore
    instructions inserted just before the offending instruction on the same
    engine queue (in-order execution preserves semantics exactly)."""
    from concourse import mybir

    n = 0
    for fn in nc.m.functions:
        for bb in fn.blocks:
            out = []
            for inst in bb.instructions:
                si = inst.sync_info
                if si is not None and si.on_wait and len(si.on_wait) > 1:
                    waits = list(si.on_wait)
                    for w in waits[:-1]:
                        n += 1
                        out.append(
                            mybir.InstEventSemaphore(
                                name=f"I-wsplit-{n}",
                                engine=inst.engine,
                                ins=[],
                                outs=[],
                                sync_info=mybir.SyncInfo(on_wait=[w], on_update=[]),
                            )
                        )
                    inst.sync_info = mybir.SyncInfo(
                        on_wait=[waits[-1]], on_update=list(si.on_update)
                    )
                out.append(inst)
            bb.instructions = out


def _prep_consts(W_ih0, W_hh0, b_ih0, b_hh0, W_ih1, W_hh1, b_ih1, b_hh1, W_lin, b_lin):
    f = np.float32
    b0 = (b_ih0 + b_hh0).astype(f)
    b1 = (b_ih1 + b_hh1).astype(f)
    cst = np.zeros((128, C_COLS), f)
    for j, g in enumerate(GATE_ORDER):
        m = 2.0 if g == G_GATE else 1.0
        cst[0:I, C_WIH0 + j * H0 : C_WIH0 + (j + 1) * H0] = (
            m * W_ih0[g * H0 : (g + 1) * H0].T
        )
        cst[I, C_WIH0 + j * H0 : C_WIH0 + (j + 1) * H0] = m * b0[g * H0 : (g + 1) * H0]
        cst[0:H0, C_WHH0 + j * H0 : C_WHH0 + (j + 1) * H0] = (
            m * W_hh0[g * H0 : (g + 1) * H0].T
        )
        # L1 chunks are zero-padded to 128 output partitions (cols j*128..j*128+H1
        # hold the real weights) so the matmuls issue at full rate.
        cst[0:H0, C_WIH1 + j * 128 : C_WIH1 + j * 128 + H1] = (
            m * W_ih1[g * H1 : (g + 1) * H1].T
        )
        cst[0:H1, C_WHH1 + j * 128 : C_WHH1 + j * 128 + H1] = (
            m * W_hh1[g * H1 : (g + 1) * H1].T
        )
        cst[H1, C_WHH1 + j * 128 : C_WHH1 + j * 128 + H1] = m * b1[g * H1 : (g + 1) * H1]
    cst[0:H1, C_WLIN] = W_lin[0]
    cst[H1, C_WLIN] = b_lin[0]
    return cst


def kernel(
    input_seq,
    mask_h0,
    mask_c0,
    mask_h1,
    mask_c1,
    W_ih0,
    W_hh0,
    b_ih0,
    b_hh0,
    W_ih1,
    W_hh1,
    b_ih1,
    b_hh1,
    W_lin,
    b_lin,
):
    import sys

    for p in ("/opt/trn_rl_repo",):
        if p not in sys.path:
            sys.path.insert(0, p)
    from concourse.bass_utils import run_bass_kernel_spmd
    import ml_dtypes

    f = np.float32
    bf = ml_dtypes.bfloat16
    input_seq = np.asarray(input_seq, f)
    mask_h0 = np.asarray(mask_h0, f)
    mask_c0 = np.asarray(mask_c0, f)
    mask_h1 = np.asarray(mask_h1, f)
    mask_c1 = np.asarray(mask_c1, f)
    args = [np.asarray(a, f) for a in (W_ih0, W_hh0, b_ih0, b_hh0,
                                       W_ih1, W_hh1, b_ih1, b_hh1, W_lin, b_lin)]
    cst = _prep_consts(*args).astype(bf)
    scale = np.float32(MASK_SCALE)

    in_maps = []
    for c in range(NCORES):
        lo, hi = c * BS, (c + 1) * BS
        xs = input_seq[lo:hi]  # [BS, T, I]
        xt = np.empty((I + 1, T * BS), bf)
        xt[0:I] = xs.transpose(2, 1, 0).reshape(I, T * BS).astype(bf)
        xt[I] = 1.0
        mph3 = np.zeros((T + LAG, 128, 128), bf)
        mpc3 = np.zeros((T + LAG, 128, 128), bf)
        mph3[0:T, :, 0:64] = (
            ((mask_h0[:, lo:hi, :] != 0) * scale).transpose(0, 2, 1).astype(bf)
        )
        mph3[LAG : T + LAG, 0:H1, 64:128] = (
            ((mask_h1[:, lo:hi, :] != 0) * scale).transpose(0, 2, 1).astype(bf)
        )
        mpc3[0:T, :, 0:64] = (
            ((mask_c0[:, lo:hi, :] != 0) * scale).transpose(0, 2, 1).astype(bf)
        )
        mpc3[LAG : T + LAG, 0:H1, 64:128] = (
            ((mask_c1[:, lo:hi, :] != 0) * scale).transpose(0, 2, 1).astype(bf)
        )
        mph = np.ascontiguousarray(mph3.transpose(1, 0, 2).reshape(128, -1))
        mpc = np.ascontiguousarray(mpc3.transpose(1, 0, 2).reshape(128, -1))
        in_maps.append({"cst": cst, "xt": xt, "mph": mph, "mpc": mpc})

    if "nc" not in _CACHED:
        _CACHED["nc"] = _build_program()
    _CACHED["in_maps"] = in_maps
    res = run_bass_kernel_spmd(_CACHED["nc"], in_maps, list(range(NCORES)))
    out = np.concatenate([res.results[c]["y"] for c in range(NCORES)], axis=0)
    return out.astype(f)
